# revision 4
# baseline (speedup 1.0000x reference)
"""MDLSTM (4-direction 2D-LSTM) Trainium2 kernel, v3.

Sharding: 8 cores = 4 scan directions x 2 batch halves (B_local=16).
Anti-diagonal wavefronts; cells (i, j) with i+j = t are independent and
depend only on wavefront t-1.

v3 changes vs v2 (TimelineSim 0.888 ms baseline):
  - steady state was ScalE(99%)+DVE(98%)-bound: per-instruction overhead
    cut with nck=3 chunks (was 4) via a single manually-offset PSUM tile
    [O, 2, 4, 512] that exactly tiles the 8 PSUM banks, so any chunk
    split of the 512 wavefront columns fits.
  - the two branches' f*c products merged into ONE tensor_tensor via an
    overlapping strided AP on the state tile (branch1 = branch0 + B).
  - state commit split into c-half and h-half: the h-commit (which gates
    the next wavefront's matmuls) no longer waits for the c-half; the
    c-half (E_c + commit_c) runs on the idle GPSIMD/Pool engine.
  - ramp wavefronts are latency-cycle-bound, not work-bound: they now
    use SMALLER chunks (shorter PE->sigma->cn->tanh->pp->commit chain
    per wavefront) instead of one big chunk.
  - x upload split into 16 DMAs so the first wavefronts start sooner.
"""

import numpy as np

B_FULL, CIN, H, W = 32, 16, 32, 128
O = 128
B = 16  # batch per core
N_CORES = 8
NG = 4  # gates i, f, o, g
WCOLS = H * B  # max wavefront columns (512)


def _wavefronts(h, w):
    out = []
    off = 0
    for t in range(h + w - 1):
        i0 = max(0, t - (w - 1))
        i1 = min(h, t + 1)
        out.append((t, i0, i1, off))
        off += (i1 - i0) * B
    return out


def _chunk_rows(d):
    """Row counts per chunk for a wavefront of d rows.

    Steady wavefronts want few big chunks (less per-instruction
    overhead); ramp wavefronts are latency-bound and want small chunks.
    """
    if d <= 3:
        return [d]
    if d <= 7:
        a = -(-d // 2)
        return [a, d - a]
    n = 3
    base = d // n
    rem = d - base * n
    return [base + (1 if k < rem else 0) for k in range(n)]


def build_module(h, w):
    import bass_rust
    import concourse.bacc as bacc
    import concourse.mybir as mybir
    import concourse.tile as tile

    dt = mybir.dt
    f16 = dt.float16
    f32 = dt.float32
    AF = mybir.ActivationFunctionType
    ALU = mybir.AluOpType

    wfs = _wavefronts(h, w)
    ncols = h * w * B
    nslots = h + 1
    wcols = h * B

    nc = bacc.Bacc("TRN2", target_bir_lowering=False, debug=False)

    x_diag = nc.dram_tensor("x_diag", [CIN + 1, ncols], f16, kind="ExternalInput")
    whT = nc.dram_tensor("whT", [O, NG * O], f16, kind="ExternalInput")
    wxT = nc.dram_tensor("wxT", [CIN + 1, NG * O], f16, kind="ExternalInput")
    ws0v = nc.dram_tensor("ws0v", [O, 1], f32, kind="ExternalInput")
    ws1v = nc.dram_tensor("ws1v", [O, 1], f32, kind="ExternalInput")
    biasv = nc.dram_tensor("biasv", [O, 1], f32, kind="ExternalInput")
    zerov = nc.dram_tensor("zerov", [O, 2, 2, nslots, B], f16, kind="ExternalInput")
    h_diag = nc.dram_tensor("h_diag", [O, ncols], f16, kind="ExternalOutput")

    with tile.TileContext(nc) as tc:
        with (
            tc.tile_pool(name="const", bufs=1) as cpool,
            tc.tile_pool(name="state", bufs=1) as spool,
            tc.tile_pool(name="gates", bufs=2) as gpool,
            tc.tile_pool(name="work", bufs=2) as wpool,
            tc.tile_pool(name="psum", bufs=1, space="PSUM") as ppool,
        ):
            whT_s = cpool.tile([O, NG * O], f16, tag="whT")
            wxT_s = cpool.tile([CIN + 1, NG * O], f16, tag="wxT")
            ws0_s = cpool.tile([O, 1], f32, tag="ws0")
            ws1_s = cpool.tile([O, 1], f32, tag="ws1")
            bias_s = cpool.tile([O, 1], f32, tag="bias")
            nc.sync.dma_start(whT_s[:], whT[:])
            nc.sync.dma_start(wxT_s[:], wxT[:])
            nc.sync.dma_start(ws0_s[:], ws0v[:])
            nc.sync.dma_start(ws1_s[:], ws1v[:])
            nc.sync.dma_start(bias_s[:], biasv[:])

            # state: [buf][c|h][slot][b]; slot 0 stays zero forever
            sc = spool.tile([O, 2, 2, nslots, B], f16, tag="sc")
            nc.sync.dma_start(sc[:], zerov[:])

            # whole x resident in SBUF; chunked DMAs so early columns land first
            xs = cpool.tile([CIN + 1, ncols], f16, tag="xs")
            nxc = 16
            xstep = -(-ncols // nxc)
            for c in range(nxc):
                lo = c * xstep
                hi = min(ncols, lo + xstep)
                nc.sync.dma_start(xs[:, lo:hi], x_diag[:, lo:hi])

            # one PSUM tile = exactly the 8 banks: plane (branch, gate) is one
            # 2KB bank; chunks are column windows of the 512 wavefront columns
            P = ppool.tile([O, 2, 4, wcols], f32, tag="P")

            # state-tile element strides for the overlapping branch view
            sc_pstride = 2 * 2 * nslots * B

            def c_overlap(bp, r0, ck):
                # [O, 2, ck] view of c-state: branch0 at slots r0.., branch1
                # shifted one slot (+B elements); reads overlap on purpose.
                base = sc[:, bp, 0, r0, 0]
                return bass_rust.AP(
                    base.tensor, base.offset, [[sc_pstride, O], [B, 2], [1, ck]]
                )

            # gate order [i, f, o, g]; psum/gate plane = 2*gi + branch
            GI, GF, GO, GG = 0, 1, 2, 3
            plane_of = {GG: 0, GI: 1, GF: 2, GO: 3}

            for t, i0, i1, off in wfs:
                d = i1 - i0
                bp = (t + 1) % 2  # prev state buffer
                bc = t % 2
                db = d * B
                rows = _chunk_rows(d)
                nck = len(rows)

                G = gpool.tile([O, 2, 4, wcols], f16, tag="G")
                CP = wpool.tile([O, 4, wcols], f16, tag="cp")
                TAU = wpool.tile([O, 2, wcols], f16, tag="tau")

                r0 = i0
                o1 = 0
                for k in range(nck):
                    dk = rows[k]
                    ck = dk * B
                    offk = off + o1
                    r1 = r0 + dk

                    xr = xs[:, offk : offk + ck]
                    rhs_t = sc[:, bp, 1, r0:r1, :]
                    rhs_l = sc[:, bp, 1, r0 + 1 : r1 + 1, :]
                    pw = slice(o1, o1 + ck)

                    for g in (GG, GI, GF, GO):
                        p = plane_of[g]
                        lx = wxT_s[:, g * O : (g + 1) * O]
                        lw = whT_s[:, g * O : (g + 1) * O]
                        nc.tensor.matmul(P[:, 0, p, pw], lx, xr, start=True, stop=False)
                        nc.tensor.matmul(P[:, 1, p, pw], lx, xr, start=True, stop=False)
                        nc.tensor.matmul(
                            P[:, 0, p, pw], lw, rhs_t, start=False, stop=True
                        )
                        nc.tensor.matmul(
                            P[:, 1, p, pw], lw, rhs_l, start=False, stop=True
                        )

                    # in latency-bound single-chunk wavefronts split out the
                    # o-gate sigmoid (only needed at pp) so it overlaps the
                    # DVE cn work; elsewhere one merged sigmoid
                    if nck == 1:
                        nc.scalar.activation(
                            G[:, :, 0:3, pw], P[:, :, 0:3, pw], AF.Sigmoid
                        )
                        nc.scalar.activation(G[:, :, 3, pw], P[:, :, 3, pw], AF.Sigmoid)
                    else:
                        nc.scalar.activation(
                            G[:, :, 0:4, pw], P[:, :, 0:4, pw], AF.Sigmoid
                        )

                    T1 = wpool.tile([O, 2, ck], f16, tag=f"t1{k}")
                    T2 = wpool.tile([O, 2, ck], f16, tag=f"t2{k}")
                    Eh = wpool.tile([O, ck], f16, tag=f"eh{k}")
                    Ec = wpool.tile([O, ck], f16, tag=f"ec{k}")

                    # g = 2*sigmoid(2x) - 1 affine fix, in place
                    nc.vector.tensor_scalar(
                        G[:, :, 0, pw], G[:, :, 0, pw], 2.0, -1.0, ALU.mult, ALU.add
                    )
                    # t2 = i*g (both branches in one op)
                    nc.vector.tensor_tensor(
                        T2[:, :, :], G[:, :, 1, pw], G[:, :, 0, pw], ALU.mult
                    )
                    # t1 = f*c_pred, both branches via the overlapping view;
                    # on the idle Pool engine for big wavefronts (it runs
                    # concurrently with gfix/t2 on the DVE), on the DVE in
                    # latency-bound ramp wavefronts
                    t1_eng = nc.gpsimd if d >= 16 else nc.vector
                    t1_eng.tensor_tensor(
                        T1[:, :, :], G[:, :, 2, pw], c_overlap(bp, r0, ck), ALU.mult
                    )
                    # cn = t1 + t2 -> CP[0:2]
                    nc.vector.tensor_tensor(
                        CP[:, 0:2, pw], T1[:, :, :], T2[:, :, :], ALU.add
                    )
                    # tau = tanh(cn)
                    nc.scalar.activation(TAU[:, :, pw], CP[:, 0:2, pw], AF.Tanh)
                    # pp = o*tau -> CP[2:4]
                    nc.vector.tensor_tensor(
                        CP[:, 2:4, pw], G[:, :, 3, pw], TAU[:, :, pw], ALU.mult
                    )
                    # h-commit first: it unblocks the next wavefront's matmuls
                    nc.vector.tensor_scalar(
                        Eh[:, :], CP[:, 2, pw], ws0_s[:], bias_s[:], ALU.mult, ALU.add
                    )
                    nc.vector.scalar_tensor_tensor(
                        sc[:, bc, 1, r0 + 1 : r1 + 1, :],
                        CP[:, 3, pw],
                        ws1_s[:],
                        Eh[:, :],
                        ALU.mult,
                        ALU.add,
                    )
                    # c-commit second (only the next wavefront's t1 needs it)
                    nc.vector.tensor_scalar(
                        Ec[:, :], CP[:, 0, pw], ws0_s[:], bias_s[:], ALU.mult, ALU.add
                    )
                    nc.vector.scalar_tensor_tensor(
                        sc[:, bc, 0, r0 + 1 : r1 + 1, :],
                        CP[:, 1, pw],
                        ws1_s[:],
                        Ec[:, :],
                        ALU.mult,
                        ALU.add,
                    )

                    r0 = r1
                    o1 += ck

                # stream this wavefront's h out in one DMA
                nc.sync.dma_start(
                    h_diag[:, off : off + db],
                    sc[:, bc, 1, i0 + 1 : i1 + 1, :],
                )

    nc.compile()
    return nc


# ---------------------------------------------------------------- host side


def _diag_index(h, w):
    cells = []
    for t, i0, i1, _ in _wavefronts(h, w):
        for i in range(i0, i1):
            cells.append((i, t - i))
    return np.array(cells)


def _prep_core_inputs(inputs, d, half, h, w):
    flips = [(False, False), (False, True), (True, False), (True, True)]
    fy, fx = flips[d]
    xd = inputs["x"][half * B : (half + 1) * B]  # (B, CIN, H, W)
    if fy:
        xd = xd[:, :, ::-1, :]
    if fx:
        xd = xd[:, :, :, ::-1]
    x_hw = np.ascontiguousarray(np.transpose(xd, (1, 2, 3, 0)))  # (CIN, H, W, B)

    cells = _diag_index(h, w)
    x_cells = x_hw[:, cells[:, 0], cells[:, 1], :].reshape(CIN, h * w * B)
    x_diag = np.ones((CIN + 1, h * w * B), np.float16)
    x_diag[:CIN] = x_cells.astype(np.float16)

    # gate order [i, f, o, g]
    gw_h = [inputs["w_hi"][d], inputs["w_hf"][d], inputs["w_ho"][d], inputs["w_hg"][d]]
    gw_x = [inputs["w_ii"][d], inputs["w_if"][d], inputs["w_io"][d], inputs["w_ig"][d]]
    gb = [inputs["b_i"][d], inputs["b_f"][d], inputs["b_o"][d], inputs["b_g"][d]]

    # gate g (block 3) weights doubled: kernel computes tanh via 2*sig(2x)-1
    whT = np.concatenate(
        [wh.T * (2.0 if g == 3 else 1.0) for g, wh in enumerate(gw_h)], axis=1
    ).astype(np.float16)
    wxT = np.zeros((CIN + 1, NG * O), np.float16)
    for g in range(NG):
        s = 2.0 if g == 3 else 1.0
        wxT[:CIN, g * O : (g + 1) * O] = (gw_x[g].T * s).astype(np.float16)
        wxT[CIN, g * O : (g + 1) * O] = (gb[g] * s).astype(np.float16)

    ws = inputs["weighted_sum"][d]
    return {
        "x_diag": x_diag,
        "whT": whT,
        "wxT": wxT,
        "ws0v": np.full((O, 1), ws[0], np.float32),
        "ws1v": np.full((O, 1), ws[1], np.float32),
        "biasv": np.asarray(inputs["bias"][d], np.float32).reshape(O, 1),
        "zerov": np.zeros((O, 2, 2, h + 1, B), np.float16),
    }


def _assemble_output(results, h, w):
    flips = [(False, False), (False, True), (True, False), (True, True)]
    cells = _diag_index(h, w)
    inv = np.empty(h * w, np.int64)
    inv[cells[:, 0] * w + cells[:, 1]] = np.arange(h * w)

    out = np.empty((NG, O, B_FULL, h, w), np.float32)
    for d in range(4):
        fy, fx = flips[d]
        for half in range(2):
            hd = results[d * 2 + half]["h_diag"].astype(np.float32)
            hv = hd.reshape(O, h * w, B)[:, inv, :].reshape(O, h, w, B)
            if fy:
                hv = hv[:, ::-1, :, :]
            if fx:
                hv = hv[:, :, ::-1, :]
            out[d, :, half * B : (half + 1) * B] = np.transpose(hv, (0, 3, 1, 2))
    return out


_module_cache = {}


def _get_module(h=H, w=W):
    key = (h, w)
    if key not in _module_cache:
        _module_cache[key] = build_module(h, w)
    return _module_cache[key]


def make_in_maps(inputs, h=H, w=W):
    return [
        _prep_core_inputs(inputs, core // 2, core % 2, h, w) for core in range(N_CORES)
    ]


def kernel(**inputs) -> np.ndarray:
    from concourse import bass_utils

    nc = _get_module(H, W)
    in_maps = make_in_maps(inputs)
    res = bass_utils.run_bass_kernel_spmd(nc, in_maps, core_ids=list(range(N_CORES)))
    return _assemble_output(res.results, H, W)


# revision 19
# speedup vs baseline: 1.5130x; 1.5130x over previous
"""MDLSTM (4-direction 2D-LSTM) Trainium2 kernel, v3.

Sharding: 8 cores = 4 scan directions x 2 batch halves (B_local=16).
Anti-diagonal wavefronts; cells (i, j) with i+j = t are independent and
depend only on wavefront t-1.

v3 changes vs v2 (TimelineSim 0.888 ms baseline):
  - steady state was ScalE(99%)+DVE(98%)-bound: per-instruction overhead
    cut with nck=3 chunks (was 4) via a single manually-offset PSUM tile
    [O, 2, 4, 512] that exactly tiles the 8 PSUM banks, so any chunk
    split of the 512 wavefront columns fits.
  - the two branches' f*c products merged into ONE tensor_tensor via an
    overlapping strided AP on the state tile (branch1 = branch0 + B).
  - state commit split into c-half and h-half: the h-commit (which gates
    the next wavefront's matmuls) no longer waits for the c-half; the
    c-half (E_c + commit_c) runs on the idle GPSIMD/Pool engine.
  - ramp wavefronts are latency-cycle-bound, not work-bound: they now
    use SMALLER chunks (shorter PE->sigma->cn->tanh->pp->commit chain
    per wavefront) instead of one big chunk.
  - x upload split into 16 DMAs so the first wavefronts start sooner.
"""

import numpy as np

B_FULL, CIN, H, W = 32, 16, 32, 128
O = 128
B = 16  # batch per core
N_CORES = 8
NG = 4  # gates i, f, o, g
WCOLS = H * B  # max wavefront columns (512)


def _wavefronts(h, w):
    out = []
    off = 0
    for t in range(h + w - 1):
        i0 = max(0, t - (w - 1))
        i1 = min(h, t + 1)
        out.append((t, i0, i1, off))
        off += (i1 - i0) * B
    return out


def _chunk_rows(d):
    """Row counts per chunk for a wavefront of d rows.

    Steady wavefronts want few big chunks (less per-instruction
    overhead); ramp wavefronts are latency-bound and want small chunks.
    """
    if d <= 3:
        return [d]
    if d <= 7:
        a = -(-d // 2)
        return [a, d - a]
    n = 3 if d <= 24 else 4
    base = d // n
    rem = d - base * n
    return [base + (1 if k < rem else 0) for k in range(n)]


def build_module(h, w):
    import bass_rust
    import concourse.bacc as bacc
    import concourse.mybir as mybir
    import concourse.tile as tile

    dt = mybir.dt
    f16 = dt.float16
    f32 = dt.float32
    AF = mybir.ActivationFunctionType
    ALU = mybir.AluOpType

    wfs = _wavefronts(h, w)
    ncols = h * w * B
    nslots = h + 1
    wcols = h * B

    nc = bacc.Bacc("TRN2", target_bir_lowering=False, debug=False)

    x_diag = nc.dram_tensor("x_diag", [CIN + 1, ncols], f16, kind="ExternalInput")
    whT = nc.dram_tensor("whT", [O, NG * O], f16, kind="ExternalInput")
    wxT = nc.dram_tensor("wxT", [CIN + 1, NG * O], f16, kind="ExternalInput")
    ws0v = nc.dram_tensor("ws0v", [O, 1], f32, kind="ExternalInput")
    ws1v = nc.dram_tensor("ws1v", [O, 1], f32, kind="ExternalInput")
    biasv = nc.dram_tensor("biasv", [O, 1], f32, kind="ExternalInput")
    zerov = nc.dram_tensor("zerov", [O, 2, 2, nslots, B], f16, kind="ExternalInput")
    h_diag = nc.dram_tensor("h_diag", [O, ncols], f16, kind="ExternalOutput")

    with tile.TileContext(nc) as tc:
        with (
            tc.tile_pool(name="const", bufs=1) as cpool,
            tc.tile_pool(name="state", bufs=1) as spool,
            tc.tile_pool(name="gates", bufs=2) as gpool,
            tc.tile_pool(name="work", bufs=2) as wpool,
            tc.tile_pool(name="psum", bufs=1, space="PSUM") as ppool,
        ):
            whT_s = cpool.tile([O, NG * O], f16, tag="whT")
            wxT_s = cpool.tile([CIN + 1, NG * O], f16, tag="wxT")
            ws0_s = cpool.tile([O, 1], f32, tag="ws0")
            ws1_s = cpool.tile([O, 1], f32, tag="ws1")
            bias_s = cpool.tile([O, 1], f32, tag="bias")
            nc.sync.dma_start(whT_s[:], whT[:])
            nc.sync.dma_start(wxT_s[:], wxT[:])
            nc.sync.dma_start(ws0_s[:], ws0v[:])
            nc.sync.dma_start(ws1_s[:], ws1v[:])
            nc.sync.dma_start(bias_s[:], biasv[:])

            # state: [buf][c|h][slot][b]; slot 0 stays zero forever.
            # Zero-initialized by memset on the (otherwise idle) Pool engine
            # so the DMA queue only carries weights + x at startup.
            sc = spool.tile([O, 2, 2, nslots, B], f16, tag="sc")
            nc.gpsimd.memset(sc[:], 0.0)

            # whole x resident in SBUF; geometric DMA chunks so the first
            # wavefronts' columns land in ~1us instead of after the full x
            xs = cpool.tile([CIN + 1, ncols], f16, tag="xs")
            bounds = [0, 512, 1536, 3584, 8192, 16384, 28672, 45056, ncols]
            for lo, hi in zip(bounds, bounds[1:]):
                nc.sync.dma_start(xs[:, lo:hi], x_diag[:, lo:hi])

            # per-chunk-slot PSUM/gate tile widths; PSUM tiles round up to
            # 2KB banks: four 4KB slots fill the 16KB PSUM exactly
            CKS = [8 * B, 8 * B, 8 * B, 8 * B]

            # state-tile element strides for the overlapping branch view
            sc_pstride = 2 * 2 * nslots * B

            def c_overlap(bp, r0, ck):
                # [O, 2, ck] view of c-state: branch0 at slots r0.., branch1
                # shifted one slot (+B elements); reads overlap on purpose.
                base = sc[:, bp, 0, r0, 0]
                return bass_rust.AP(
                    base.tensor, base.offset, [[sc_pstride, O], [B, 2], [1, ck]]
                )

            # gate order [i, f, o, g]; psum/gate plane = 2*gi + branch
            GI, GF, GO, GG = 0, 1, 2, 3
            plane_of = {GG: 0, GI: 1, GF: 2, GO: 3}

            for t, i0, i1, off in wfs:
                d = i1 - i0
                bp = (t + 1) % 2  # prev state buffer
                bc = t % 2
                db = d * B
                rows = _chunk_rows(d)
                nck = len(rows)

                CP = wpool.tile([O, 4, wcols], f16, tag="cp")
                TAU = wpool.tile([O, 2, wcols], f16, tag="tau")

                # chunk metadata: (k, row0, row1, ck, wavefront col offset)
                cks = []
                r0, o1 = i0, 0
                for k, dk in enumerate(rows):
                    cks.append((k, r0, r0 + dk, dk * B, o1))
                    r0 += dk
                    o1 += dk * B

                # In big (throughput-bound) wavefronts interleave sigma/tanh
                # per chunk: ScalE is backlogged, so tanh never stalls it.
                # Mid-ramp wavefronts are latency-bound with an idle ScalE:
                # issuing every sigma before the first tanh removes the
                # head-of-line stall of tanh(k) waiting on the DVE's cn(k).
                reorder = 2 <= nck <= 3

                def mm_sigma(k, cr0, cr1, ck, co1):
                    xr = xs[:, off + co1 : off + co1 + ck]
                    rhs_t = sc[:, bp, 1, cr0:cr1, :]
                    rhs_l = sc[:, bp, 1, cr0 + 1 : cr1 + 1, :]
                    P = ppool.tile([O, 2, 4, CKS[k]], f32, tag=f"p{k}")
                    G = gpool.tile([O, 2, 4, CKS[k]], f16, tag=f"g{k}")
                    for g in (GG, GI, GF, GO):
                        p = plane_of[g]
                        lx = wxT_s[:, g * O : (g + 1) * O]
                        lw = whT_s[:, g * O : (g + 1) * O]
                        nc.tensor.matmul(
                            P[:, 0, p, :ck], lx, xr, start=True, stop=False
                        )
                        nc.tensor.matmul(
                            P[:, 1, p, :ck], lx, xr, start=True, stop=False
                        )
                        nc.tensor.matmul(
                            P[:, 0, p, :ck], lw, rhs_t, start=False, stop=True
                        )
                        nc.tensor.matmul(
                            P[:, 1, p, :ck], lw, rhs_l, start=False, stop=True
                        )
                    # in latency-bound single-chunk wavefronts split out the
                    # o-gate sigmoid (only needed at pp) so it overlaps the
                    # DVE cn work; elsewhere one merged sigmoid
                    if nck == 1:
                        nc.scalar.activation(
                            G[:, :, 0:3, :ck], P[:, :, 0:3, :ck], AF.Sigmoid
                        )
                        nc.scalar.activation(G[:, :, 3, :ck], P[:, :, 3, :ck], AF.Sigmoid)
                    else:
                        nc.scalar.activation(
                            G[:, :, 0:4, :ck], P[:, :, 0:4, :ck], AF.Sigmoid
                        )
                    return G

                def cn_stage(k, cr0, cr1, ck, co1, G):
                    pw = slice(co1, co1 + ck)
                    T1 = wpool.tile([O, 2, CKS[k]], f16, tag=f"t1{k}")
                    T2 = wpool.tile([O, 2, CKS[k]], f16, tag=f"t2{k}")

                    # g = 2*sigmoid(2x) - 1 affine fix, in place
                    nc.vector.tensor_scalar(
                        G[:, :, 0, :ck], G[:, :, 0, :ck], 2.0, -1.0, ALU.mult, ALU.add
                    )
                    # t2 = i*g (both branches in one op)
                    nc.vector.tensor_tensor(
                        T2[:, :, :ck], G[:, :, 1, :ck], G[:, :, 0, :ck], ALU.mult
                    )
                    # t1 = f*c_pred, both branches via the overlapping view
                    nc.vector.tensor_tensor(
                        T1[:, :, :ck], G[:, :, 2, :ck], c_overlap(bp, cr0, ck), ALU.mult
                    )
                    # cn = t1 + t2 -> CP[0:2]
                    nc.vector.tensor_tensor(
                        CP[:, 0:2, pw], T1[:, :, :ck], T2[:, :, :ck], ALU.add
                    )

                def tanh_stage(c0, c1):
                    # one tanh covering chunks c0..c1 (they wrote disjoint
                    # column ranges of CP): ScalE overhead paid once
                    lo = cks[c0][4]
                    hi = cks[c1][4] + cks[c1][3]
                    nc.scalar.activation(
                        TAU[:, :, lo:hi], CP[:, 0:2, lo:hi], AF.Tanh
                    )

                def commit_stage(k, cr0, cr1, ck, co1, G):
                    pw = slice(co1, co1 + ck)
                    Eh = wpool.tile([O, 2, CKS[k]], f16, tag=f"eh{k}")
                    # pp = o*tau -> CP[2:4]; on the idle Pool engine in big
                    # wavefronts (throughput-bound), on the DVE in
                    # latency-bound ramp wavefronts
                    pp_eng = nc.gpsimd if d >= 32 else nc.vector
                    pp_eng.tensor_tensor(
                        CP[:, 2:4, pw], G[:, :, 3, :ck], TAU[:, :, pw], ALU.mult
                    )
                    # E = ws0*[cn_t, pp_t] + bias, then
                    # new state [ct | ht] = ws1*[cn_l, pp_l] + E in one op
                    nc.vector.tensor_scalar(
                        Eh[:, :, :ck],
                        CP[:, 0:4:2, pw],
                        ws0_s[:],
                        bias_s[:],
                        ALU.mult,
                        ALU.add,
                    )
                    nc.vector.scalar_tensor_tensor(
                        sc[:, bc, 0:2, cr0 + 1 : cr1 + 1, :],
                        CP[:, 1:4:2, pw],
                        ws1_s[:],
                        Eh[:, :, :ck],
                        ALU.mult,
                        ALU.add,
                    )

                if reorder:
                    # mid-ramp: all sigmas back-to-back (idle ScalE never
                    # stalls on the DVE), then per-chunk cn/tanh/commit
                    Gs = [mm_sigma(*c) for c in cks]
                    for c, G in zip(cks, Gs):
                        cn_stage(*c, G)
                        tanh_stage(c[0], c[0])
                        commit_stage(*c, G)
                else:
                    # steady state: fully interleaved per chunk; commit(k)
                    # lands as early as possible so the next wavefront's
                    # matmuls (which gate everything) start immediately
                    for c in cks:
                        G = mm_sigma(*c)
                        cn_stage(*c, G)
                        tanh_stage(c[0], c[0])
                        commit_stage(*c, G)

                # stream this wavefront's h out in one DMA
                nc.sync.dma_start(
                    h_diag[:, off : off + db],
                    sc[:, bc, 1, i0 + 1 : i1 + 1, :],
                )

    nc.compile()
    return nc


# ---------------------------------------------------------------- host side


def _diag_index(h, w):
    cells = []
    for t, i0, i1, _ in _wavefronts(h, w):
        for i in range(i0, i1):
            cells.append((i, t - i))
    return np.array(cells)


def _prep_core_inputs(inputs, d, half, h, w):
    flips = [(False, False), (False, True), (True, False), (True, True)]
    fy, fx = flips[d]
    xd = inputs["x"][half * B : (half + 1) * B]  # (B, CIN, H, W)
    if fy:
        xd = xd[:, :, ::-1, :]
    if fx:
        xd = xd[:, :, :, ::-1]
    x_hw = np.ascontiguousarray(np.transpose(xd, (1, 2, 3, 0)))  # (CIN, H, W, B)

    cells = _diag_index(h, w)
    x_cells = x_hw[:, cells[:, 0], cells[:, 1], :].reshape(CIN, h * w * B)
    x_diag = np.ones((CIN + 1, h * w * B), np.float16)
    x_diag[:CIN] = x_cells.astype(np.float16)

    # gate order [i, f, o, g]
    gw_h = [inputs["w_hi"][d], inputs["w_hf"][d], inputs["w_ho"][d], inputs["w_hg"][d]]
    gw_x = [inputs["w_ii"][d], inputs["w_if"][d], inputs["w_io"][d], inputs["w_ig"][d]]
    gb = [inputs["b_i"][d], inputs["b_f"][d], inputs["b_o"][d], inputs["b_g"][d]]

    # gate g (block 3) weights doubled: kernel computes tanh via 2*sig(2x)-1
    whT = np.concatenate(
        [wh.T * (2.0 if g == 3 else 1.0) for g, wh in enumerate(gw_h)], axis=1
    ).astype(np.float16)
    wxT = np.zeros((CIN + 1, NG * O), np.float16)
    for g in range(NG):
        s = 2.0 if g == 3 else 1.0
        wxT[:CIN, g * O : (g + 1) * O] = (gw_x[g].T * s).astype(np.float16)
        wxT[CIN, g * O : (g + 1) * O] = (gb[g] * s).astype(np.float16)

    ws = inputs["weighted_sum"][d]
    return {
        "x_diag": x_diag,
        "whT": whT,
        "wxT": wxT,
        "ws0v": np.full((O, 1), ws[0], np.float32),
        "ws1v": np.full((O, 1), ws[1], np.float32),
        "biasv": np.asarray(inputs["bias"][d], np.float32).reshape(O, 1),
        "zerov": np.zeros((O, 2, 2, h + 1, B), np.float16),
    }


def _assemble_output(results, h, w):
    flips = [(False, False), (False, True), (True, False), (True, True)]
    cells = _diag_index(h, w)
    inv = np.empty(h * w, np.int64)
    inv[cells[:, 0] * w + cells[:, 1]] = np.arange(h * w)

    out = np.empty((NG, O, B_FULL, h, w), np.float32)
    for d in range(4):
        fy, fx = flips[d]
        for half in range(2):
            hd = results[d * 2 + half]["h_diag"].astype(np.float32)
            hv = hd.reshape(O, h * w, B)[:, inv, :].reshape(O, h, w, B)
            if fy:
                hv = hv[:, ::-1, :, :]
            if fx:
                hv = hv[:, :, ::-1, :]
            out[d, :, half * B : (half + 1) * B] = np.transpose(hv, (0, 3, 1, 2))
    return out


_module_cache = {}


def _get_module(h=H, w=W):
    key = (h, w)
    if key not in _module_cache:
        _module_cache[key] = build_module(h, w)
    return _module_cache[key]


def make_in_maps(inputs, h=H, w=W):
    return [
        _prep_core_inputs(inputs, core // 2, core % 2, h, w) for core in range(N_CORES)
    ]


def kernel(**inputs) -> np.ndarray:
    from concourse import bass_utils

    nc = _get_module(H, W)
    in_maps = make_in_maps(inputs)
    res = bass_utils.run_bass_kernel_spmd(nc, in_maps, core_ids=list(range(N_CORES)))
    return _assemble_output(res.results, H, W)


# revision 24
# speedup vs baseline: 1.5326x; 1.0130x over previous
"""MDLSTM (4-direction 2D-LSTM) Trainium2 kernel, v4.

Sharding: 8 cores = 4 scan directions x 2 batch-quarter PAIRS. Each
core runs TWO independent B=8 scans of the same direction in lockstep
(skew 0). The ramp wavefronts are latency-chain-bound, not work-bound;
two half-width scans ramp concurrently on independent dependency
chains, so the ramp wall-clock is one half-width chain instead of one
full-width chain, while steady state keeps the same instruction sizes
(4 x 128-column chunk-slots per step).

Per-scan wavefront structure (anti-diagonals; cells (i,j) with i+j=t):
  - fp16 end to end; tanh(g) via 2*sigmoid(2x)-1 with weights
    pre-doubled on host.
  - PSUM plane layout [branch][gate]; per gate the two branches'
    accumulation groups sit in different banks.
  - both branches' f*c products in ONE tensor_tensor via an
    overlapping strided AP on the state tile (branch1 = branch0 + B).
  - pp = o*tau on the Pool engine in steady state (DVE relief).
  - mid-ramp wavefronts issue both sigmas before the first tanh so the
    idle ScalE never head-of-line stalls on the DVE's cn.
  - zero-state by Pool memset; geometric x DMA chunks so the first
    wavefronts start in ~2us.
"""

import numpy as np

B_FULL, CIN, H, W = 32, 16, 32, 128
O = 128
SB = 8  # batch per scan (two scans per core)
N_CORES = 8
NG = 4  # gates i, f, o, g


def _wavefronts(h, w, b):
    out = []
    off = 0
    for t in range(h + w - 1):
        i0 = max(0, t - (w - 1))
        i1 = min(h, t + 1)
        out.append((t, i0, i1, off))
        off += (i1 - i0) * b
    return out


def _chunk_rows(d):
    if d <= 7:
        return [d]
    a = -(-d // 2)
    return [a, d - a]


def build_module(h, w):
    import bass_rust
    import concourse.bacc as bacc
    import concourse.mybir as mybir
    import concourse.tile as tile

    dt = mybir.dt
    f16 = dt.float16
    f32 = dt.float32
    AF = mybir.ActivationFunctionType
    ALU = mybir.AluOpType

    wfs = _wavefronts(h, w, SB)
    ncols = h * w * SB
    nslots = h + 1

    nc = bacc.Bacc("TRN2", target_bir_lowering=False, debug=False)

    x_diag = nc.dram_tensor("x_diag", [CIN + 1, 2 * ncols], f16, kind="ExternalInput")
    whT = nc.dram_tensor("whT", [O, NG * O], f16, kind="ExternalInput")
    wxT = nc.dram_tensor("wxT", [CIN + 1, NG * O], f16, kind="ExternalInput")
    ws0v = nc.dram_tensor("ws0v", [O, 1], f32, kind="ExternalInput")
    ws1v = nc.dram_tensor("ws1v", [O, 1], f32, kind="ExternalInput")
    biasv = nc.dram_tensor("biasv", [O, 1], f32, kind="ExternalInput")
    h_diag = nc.dram_tensor("h_diag", [O, 2 * ncols], f16, kind="ExternalOutput")

    with tile.TileContext(nc) as tc:
        with (
            tc.tile_pool(name="const", bufs=1) as cpool,
            tc.tile_pool(name="state", bufs=1) as spool,
            tc.tile_pool(name="gates", bufs=2) as gpool,
            tc.tile_pool(name="work", bufs=2) as wpool,
            tc.tile_pool(name="psum", bufs=1, space="PSUM") as ppool,
        ):
            whT_s = cpool.tile([O, NG * O], f16, tag="whT")
            wxT_s = cpool.tile([CIN + 1, NG * O], f16, tag="wxT")
            ws0_s = cpool.tile([O, 1], f32, tag="ws0")
            ws1_s = cpool.tile([O, 1], f32, tag="ws1")
            bias_s = cpool.tile([O, 1], f32, tag="bias")
            nc.sync.dma_start(whT_s[:], whT[:])
            nc.sync.dma_start(wxT_s[:], wxT[:])
            nc.sync.dma_start(ws0_s[:], ws0v[:])
            nc.sync.dma_start(ws1_s[:], ws1v[:])
            nc.sync.dma_start(bias_s[:], biasv[:])

            # per-scan state: [buf][c|h][slot][b]; slot 0 stays zero forever.
            # Zeroed by memset on the (otherwise idle) Pool engine.
            scs = []
            for sn in range(2):
                t_ = spool.tile([O, 2, 2, nslots, SB], f16, tag=f"sc{sn}")
                nc.gpsimd.memset(t_[:], 0.0)
                scs.append(t_)

            # both scans' x resident in SBUF; geometric DMA chunks per scan
            # (interleaved) so both scans' first wavefronts start in ~2us
            xs = cpool.tile([CIN + 1, 2 * ncols], f16, tag="xs")
            bounds = [0, 512, 1536, 3584, 8192, 16384, 24576, ncols]
            for lo, hi in zip(bounds, bounds[1:]):
                for sn in range(2):
                    b0 = sn * ncols
                    nc.sync.dma_start(
                        xs[:, b0 + lo : b0 + hi], x_diag[:, b0 + lo : b0 + hi]
                    )

            # 4 chunk-slots (2 per scan) of [O, 2, 4, 128] f32 = 4KB each
            # fill the 16KB PSUM exactly
            CKMAX = 16 * SB

            sc_pstride = 2 * 2 * nslots * SB

            def c_overlap(sn, bp, r0, ck):
                # [O, 2, ck] view of c-state: branch0 at slots r0.., branch1
                # shifted one slot (+SB elements); reads overlap on purpose.
                base = scs[sn][:, bp, 0, r0, 0]
                return bass_rust.AP(
                    base.tensor, base.offset, [[sc_pstride, O], [SB, 2], [1, ck]]
                )

            # gate order [i, f, o, g]; psum/gate plane = 2*gi + branch
            GI, GF, GO, GG = 0, 1, 2, 3
            plane_of = {GG: 0, GI: 1, GF: 2, GO: 3}

            for t, i0, i1, off in wfs:
                d = i1 - i0
                bp = (t + 1) % 2  # prev state buffer
                bc = t % 2
                db = d * SB
                rows = _chunk_rows(d)
                nck = len(rows)

                # chunk metadata: (k, row0, row1, ck, wavefront col offset)
                cks = []
                r0, o1 = i0, 0
                for k, dk in enumerate(rows):
                    cks.append((k, r0, r0 + dk, dk * SB, o1))
                    r0 += dk
                    o1 += dk * SB

                # Big wavefronts interleave sigma/tanh per chunk (backlogged
                # ScalE never stalls); latency-bound mid-ramp wavefronts
                # issue both sigmas before the first tanh.
                reorder = nck == 2 and d <= 30

                def scan_ctx(sn):
                    sc = scs[sn]
                    xbase = sn * ncols + off
                    CP = wpool.tile([O, 4, 2 * CKMAX], f16, tag=f"cp{sn}")
                    TAU = wpool.tile([O, 2, 2 * CKMAX], f16, tag=f"tau{sn}")

                    def mm_sigma(k, cr0, cr1, ck, co1):
                        sl = sn * 2 + k
                        xr = xs[:, xbase + co1 : xbase + co1 + ck]
                        rhs_t = sc[:, bp, 1, cr0:cr1, :]
                        rhs_l = sc[:, bp, 1, cr0 + 1 : cr1 + 1, :]
                        P = ppool.tile([O, 2, 4, CKMAX], f32, tag=f"p{sl}")
                        G = gpool.tile([O, 2, 4, CKMAX], f16, tag=f"g{sl}")
                        for g in (GG, GI, GF, GO):
                            p = plane_of[g]
                            lx = wxT_s[:, g * O : (g + 1) * O]
                            lw = whT_s[:, g * O : (g + 1) * O]
                            nc.tensor.matmul(
                                P[:, 0, p, :ck], lx, xr, start=True, stop=False
                            )
                            nc.tensor.matmul(
                                P[:, 1, p, :ck], lx, xr, start=True, stop=False
                            )
                            nc.tensor.matmul(
                                P[:, 0, p, :ck], lw, rhs_t, start=False, stop=True
                            )
                            nc.tensor.matmul(
                                P[:, 1, p, :ck], lw, rhs_l, start=False, stop=True
                            )
                        # single-chunk (latency-bound) wavefronts: split out
                        # the o-gate sigmoid so it overlaps the DVE cn work
                        if nck == 1:
                            nc.scalar.activation(
                                G[:, :, 0:3, :ck], P[:, :, 0:3, :ck], AF.Sigmoid
                            )
                            nc.scalar.activation(
                                G[:, :, 3, :ck], P[:, :, 3, :ck], AF.Sigmoid
                            )
                        else:
                            nc.scalar.activation(
                                G[:, :, 0:4, :ck], P[:, :, 0:4, :ck], AF.Sigmoid
                            )
                        return G

                    def cn_stage(k, cr0, cr1, ck, co1, G):
                        sl = sn * 2 + k
                        pw = slice(co1, co1 + ck)
                        T1 = wpool.tile([O, 2, CKMAX], f16, tag=f"t1{sl}")
                        T2 = wpool.tile([O, 2, CKMAX], f16, tag=f"t2{sl}")
                        # g = 2*sigmoid(2x) - 1 affine fix, in place
                        nc.vector.tensor_scalar(
                            G[:, :, 0, :ck],
                            G[:, :, 0, :ck],
                            2.0,
                            -1.0,
                            ALU.mult,
                            ALU.add,
                        )
                        # t2 = i*g (both branches in one op)
                        nc.vector.tensor_tensor(
                            T2[:, :, :ck], G[:, :, 1, :ck], G[:, :, 0, :ck], ALU.mult
                        )
                        # t1 = f*c_pred, both branches via the overlapping view
                        nc.vector.tensor_tensor(
                            T1[:, :, :ck],
                            G[:, :, 2, :ck],
                            c_overlap(sn, bp, cr0, ck),
                            ALU.mult,
                        )
                        # cn = t1 + t2 -> CP[0:2]
                        nc.vector.tensor_tensor(
                            CP[:, 0:2, pw], T1[:, :, :ck], T2[:, :, :ck], ALU.add
                        )

                    def tanh_commit(k, cr0, cr1, ck, co1, G):
                        sl = sn * 2 + k
                        pw = slice(co1, co1 + ck)
                        Eh = wpool.tile([O, 2, CKMAX], f16, tag=f"eh{sl}")
                        # tau = tanh(cn)
                        nc.scalar.activation(TAU[:, :, pw], CP[:, 0:2, pw], AF.Tanh)
                        # pp = o*tau -> CP[2:4]; Pool engine when
                        # throughput-bound, DVE when latency-bound
                        pp_eng = nc.gpsimd if d >= 25 else nc.vector
                        pp_eng.tensor_tensor(
                            CP[:, 2:4, pw], G[:, :, 3, :ck], TAU[:, :, pw], ALU.mult
                        )
                        # E = ws0*[cn_t, pp_t] + bias, then
                        # new state [ct | ht] = ws1*[cn_l, pp_l] + E in one op
                        nc.vector.tensor_scalar(
                            Eh[:, :, :ck],
                            CP[:, 0:4:2, pw],
                            ws0_s[:],
                            bias_s[:],
                            ALU.mult,
                            ALU.add,
                        )
                        nc.vector.scalar_tensor_tensor(
                            sc[:, bc, 0:2, cr0 + 1 : cr1 + 1, :],
                            CP[:, 1:4:2, pw],
                            ws1_s[:],
                            Eh[:, :, :ck],
                            ALU.mult,
                            ALU.add,
                        )

                    def dma_out():
                        nc.sync.dma_start(
                            h_diag[:, sn * ncols + off : sn * ncols + off + db],
                            sc[:, bc, 1, i0 + 1 : i1 + 1, :],
                        )

                    return mm_sigma, cn_stage, tanh_commit, dma_out

                ctxs = [scan_ctx(0), scan_ctx(1)]
                if d <= 30:
                    # ramp: emit each scan's wavefront whole (sigmas first
                    # within a scan); the sibling scan's block fills the
                    # chain stalls of this one
                    for sn in range(2):
                        mm, cn, tcm, _ = ctxs[sn]
                        Gs = [mm(*c) for c in cks]
                        for c, G in zip(cks, Gs):
                            cn(*c, G)
                            tcm(*c, G)
                else:
                    # steady: interleave the scans at chunk granularity, the
                    # sibling's mm+cn between a chunk's cn and its tanh
                    for c in cks:
                        Gs = {}
                        for sn in range(2):
                            Gs[sn] = ctxs[sn][0](*c)
                            ctxs[sn][1](*c, Gs[sn])
                        for sn in range(2):
                            ctxs[sn][2](*c, Gs[sn])
                for sn in range(2):
                    ctxs[sn][3]()

    nc.compile()
    return nc


# ---------------------------------------------------------------- host side


def _diag_index(h, w):
    cells = []
    for t, i0, i1, _ in _wavefronts(h, w, SB):
        for i in range(i0, i1):
            cells.append((i, t - i))
    return np.array(cells)


def _prep_core_inputs(inputs, core, h, w):
    flips = [(False, False), (False, True), (True, False), (True, True)]
    d = core // 2
    fy, fx = flips[d]
    cells = _diag_index(h, w)

    x_diag = np.ones((CIN + 1, 2 * h * w * SB), np.float16)
    for sn in range(2):
        q = (core % 2) * 2 + sn
        xd = inputs["x"][q * SB : (q + 1) * SB]  # (SB, CIN, H, W)
        if fy:
            xd = xd[:, :, ::-1, :]
        if fx:
            xd = xd[:, :, :, ::-1]
        x_hw = np.ascontiguousarray(np.transpose(xd, (1, 2, 3, 0)))  # (CIN,H,W,SB)
        x_cells = x_hw[:, cells[:, 0], cells[:, 1], :].reshape(CIN, h * w * SB)
        x_diag[:CIN, sn * h * w * SB : (sn + 1) * h * w * SB] = x_cells.astype(
            np.float16
        )

    # gate order [i, f, o, g]
    gw_h = [inputs["w_hi"][d], inputs["w_hf"][d], inputs["w_ho"][d], inputs["w_hg"][d]]
    gw_x = [inputs["w_ii"][d], inputs["w_if"][d], inputs["w_io"][d], inputs["w_ig"][d]]
    gb = [inputs["b_i"][d], inputs["b_f"][d], inputs["b_o"][d], inputs["b_g"][d]]

    # gate g (block 3) weights doubled: kernel computes tanh via 2*sig(2x)-1
    whT = np.concatenate(
        [wh.T * (2.0 if g == 3 else 1.0) for g, wh in enumerate(gw_h)], axis=1
    ).astype(np.float16)
    wxT = np.zeros((CIN + 1, NG * O), np.float16)
    for g in range(NG):
        s = 2.0 if g == 3 else 1.0
        wxT[:CIN, g * O : (g + 1) * O] = (gw_x[g].T * s).astype(np.float16)
        wxT[CIN, g * O : (g + 1) * O] = (gb[g] * s).astype(np.float16)

    ws = inputs["weighted_sum"][d]
    return {
        "x_diag": x_diag,
        "whT": whT,
        "wxT": wxT,
        "ws0v": np.full((O, 1), ws[0], np.float32),
        "ws1v": np.full((O, 1), ws[1], np.float32),
        "biasv": np.asarray(inputs["bias"][d], np.float32).reshape(O, 1),
    }


def _assemble_output(results, h, w):
    flips = [(False, False), (False, True), (True, False), (True, True)]
    cells = _diag_index(h, w)
    inv = np.empty(h * w, np.int64)
    inv[cells[:, 0] * w + cells[:, 1]] = np.arange(h * w)

    out = np.empty((NG, O, B_FULL, h, w), np.float32)
    for core in range(N_CORES):
        d = core // 2
        fy, fx = flips[d]
        hd = results[core]["h_diag"].astype(np.float32)
        for sn in range(2):
            q = (core % 2) * 2 + sn
            hq = hd[:, sn * h * w * SB : (sn + 1) * h * w * SB]
            hv = hq.reshape(O, h * w, SB)[:, inv, :].reshape(O, h, w, SB)
            if fy:
                hv = hv[:, ::-1, :, :]
            if fx:
                hv = hv[:, :, ::-1, :]
            out[d, :, q * SB : (q + 1) * SB] = np.transpose(hv, (0, 3, 1, 2))
    return out


_module_cache = {}


def _get_module(h=H, w=W):
    key = (h, w)
    if key not in _module_cache:
        _module_cache[key] = build_module(h, w)
    return _module_cache[key]


def make_in_maps(inputs, h=H, w=W):
    return [_prep_core_inputs(inputs, core, h, w) for core in range(N_CORES)]


def kernel(**inputs) -> np.ndarray:
    from concourse import bass_utils

    nc = _get_module(H, W)
    in_maps = make_in_maps(inputs)
    res = bass_utils.run_bass_kernel_spmd(nc, in_maps, core_ids=list(range(N_CORES)))
    return _assemble_output(res.results, H, W)


# revision 25
# speedup vs baseline: 1.5358x; 1.0021x over previous
"""MDLSTM (4-direction 2D-LSTM) Trainium2 kernel, v4.

Sharding: 8 cores = 4 scan directions x 2 batch-quarter PAIRS. Each
core runs TWO independent B=8 scans of the same direction in lockstep
(skew 0). The ramp wavefronts are latency-chain-bound, not work-bound;
two half-width scans ramp concurrently on independent dependency
chains, so the ramp wall-clock is one half-width chain instead of one
full-width chain, while steady state keeps the same instruction sizes
(4 x 128-column chunk-slots per step).

Per-scan wavefront structure (anti-diagonals; cells (i,j) with i+j=t):
  - fp16 end to end; tanh(g) via 2*sigmoid(2x)-1 with weights
    pre-doubled on host.
  - PSUM plane layout [branch][gate]; per gate the two branches'
    accumulation groups sit in different banks.
  - both branches' f*c products in ONE tensor_tensor via an
    overlapping strided AP on the state tile (branch1 = branch0 + B).
  - pp = o*tau on the Pool engine in steady state (DVE relief).
  - ramp wavefronts (d<=30) emit scan-sequentially with all sigmas
    before the first tanh (idle ScalE never head-of-line stalls on the
    DVE's cn); steady wavefronts interleave the two scans at chunk
    granularity with per-chunk tanh + immediate commits (commit(k)
    gates the next wavefront's matmuls, so it must land early).
  - zero-state by Pool memset; geometric x DMA chunks so the first
    wavefronts start early.
"""

import numpy as np

B_FULL, CIN, H, W = 32, 16, 32, 128
O = 128
SB = 8  # batch per scan (two scans per core)
N_CORES = 8
NG = 4  # gates i, f, o, g


def _wavefronts(h, w, b):
    out = []
    off = 0
    for t in range(h + w - 1):
        i0 = max(0, t - (w - 1))
        i1 = min(h, t + 1)
        out.append((t, i0, i1, off))
        off += (i1 - i0) * b
    return out


def _chunk_rows(d):
    if d <= 7:
        return [d]
    a = -(-d // 2)
    return [a, d - a]


def build_module(h, w):
    import bass_rust
    import concourse.bacc as bacc
    import concourse.mybir as mybir
    import concourse.tile as tile

    dt = mybir.dt
    f16 = dt.float16
    f32 = dt.float32
    AF = mybir.ActivationFunctionType
    ALU = mybir.AluOpType

    wfs = _wavefronts(h, w, SB)
    ncols = h * w * SB
    nslots = h + 1

    nc = bacc.Bacc("TRN2", target_bir_lowering=False, debug=False)

    x_diag = nc.dram_tensor("x_diag", [CIN + 1, 2 * ncols], f16, kind="ExternalInput")
    whT = nc.dram_tensor("whT", [O, NG * O], f16, kind="ExternalInput")
    wxT = nc.dram_tensor("wxT", [CIN + 1, NG * O], f16, kind="ExternalInput")
    ws0v = nc.dram_tensor("ws0v", [O, 1], f32, kind="ExternalInput")
    ws1v = nc.dram_tensor("ws1v", [O, 1], f32, kind="ExternalInput")
    biasv = nc.dram_tensor("biasv", [O, 1], f32, kind="ExternalInput")
    h_diag = nc.dram_tensor("h_diag", [O, 2 * ncols], f16, kind="ExternalOutput")

    with tile.TileContext(nc) as tc:
        with (
            tc.tile_pool(name="const", bufs=1) as cpool,
            tc.tile_pool(name="state", bufs=1) as spool,
            tc.tile_pool(name="gates", bufs=2) as gpool,
            tc.tile_pool(name="work", bufs=2) as wpool,
            tc.tile_pool(name="psum", bufs=1, space="PSUM") as ppool,
        ):
            whT_s = cpool.tile([O, NG * O], f16, tag="whT")
            wxT_s = cpool.tile([CIN + 1, NG * O], f16, tag="wxT")
            ws0_s = cpool.tile([O, 1], f32, tag="ws0")
            ws1_s = cpool.tile([O, 1], f32, tag="ws1")
            bias_s = cpool.tile([O, 1], f32, tag="bias")
            nc.sync.dma_start(whT_s[:], whT[:])
            nc.sync.dma_start(wxT_s[:], wxT[:])
            nc.sync.dma_start(ws0_s[:], ws0v[:])
            nc.sync.dma_start(ws1_s[:], ws1v[:])
            nc.sync.dma_start(bias_s[:], biasv[:])

            # per-scan state: [buf][c|h][slot][b]; slot 0 stays zero forever.
            # Zeroed by memset on the (otherwise idle) Pool engine.
            scs = []
            for sn in range(2):
                t_ = spool.tile([O, 2, 2, nslots, SB], f16, tag=f"sc{sn}")
                nc.gpsimd.memset(t_[:], 0.0)
                scs.append(t_)

            # both scans' x resident in SBUF; geometric DMA chunks per scan
            # (interleaved) so both scans' first wavefronts start in ~2us
            xs = cpool.tile([CIN + 1, 2 * ncols], f16, tag="xs")
            bounds = [0, 512, 1536, 3584, 8192, 16384, 24576, ncols]
            for lo, hi in zip(bounds, bounds[1:]):
                for sn in range(2):
                    b0 = sn * ncols
                    nc.sync.dma_start(
                        xs[:, b0 + lo : b0 + hi], x_diag[:, b0 + lo : b0 + hi]
                    )

            # 4 chunk-slots (2 per scan) of [O, 2, 4, 128] f32 = 4KB each
            # fill the 16KB PSUM exactly
            CKMAX = 16 * SB

            sc_pstride = 2 * 2 * nslots * SB

            def c_overlap(sn, bp, r0, ck):
                # [O, 2, ck] view of c-state: branch0 at slots r0.., branch1
                # shifted one slot (+SB elements); reads overlap on purpose.
                base = scs[sn][:, bp, 0, r0, 0]
                return bass_rust.AP(
                    base.tensor, base.offset, [[sc_pstride, O], [SB, 2], [1, ck]]
                )

            # gate order [i, f, o, g]; psum/gate plane = 2*gi + branch
            GI, GF, GO, GG = 0, 1, 2, 3
            plane_of = {GG: 0, GI: 1, GF: 2, GO: 3}

            for t, i0, i1, off in wfs:
                d = i1 - i0
                bp = (t + 1) % 2  # prev state buffer
                bc = t % 2
                db = d * SB
                rows = _chunk_rows(d)
                nck = len(rows)

                # chunk metadata: (k, row0, row1, ck, wavefront col offset)
                cks = []
                r0, o1 = i0, 0
                for k, dk in enumerate(rows):
                    cks.append((k, r0, r0 + dk, dk * SB, o1))
                    r0 += dk
                    o1 += dk * SB

                # Big wavefronts interleave sigma/tanh per chunk (backlogged
                # ScalE never stalls); latency-bound mid-ramp wavefronts
                # issue both sigmas before the first tanh.
                reorder = nck == 2 and d <= 30

                def scan_ctx(sn):
                    sc = scs[sn]
                    xbase = sn * ncols + off
                    CP = wpool.tile([O, 4, 2 * CKMAX], f16, tag=f"cp{sn}")
                    TAU = wpool.tile([O, 2, 2 * CKMAX], f16, tag=f"tau{sn}")

                    def mm_sigma(k, cr0, cr1, ck, co1):
                        sl = sn * 2 + k
                        xr = xs[:, xbase + co1 : xbase + co1 + ck]
                        rhs_t = sc[:, bp, 1, cr0:cr1, :]
                        rhs_l = sc[:, bp, 1, cr0 + 1 : cr1 + 1, :]
                        P = ppool.tile([O, 2, 4, CKMAX], f32, tag=f"p{sl}")
                        G = gpool.tile([O, 2, 4, CKMAX], f16, tag=f"g{sl}")
                        for g in (GG, GI, GF, GO):
                            p = plane_of[g]
                            lx = wxT_s[:, g * O : (g + 1) * O]
                            lw = whT_s[:, g * O : (g + 1) * O]
                            nc.tensor.matmul(
                                P[:, 0, p, :ck], lx, xr, start=True, stop=False
                            )
                            nc.tensor.matmul(
                                P[:, 1, p, :ck], lx, xr, start=True, stop=False
                            )
                            nc.tensor.matmul(
                                P[:, 0, p, :ck], lw, rhs_t, start=False, stop=True
                            )
                            nc.tensor.matmul(
                                P[:, 1, p, :ck], lw, rhs_l, start=False, stop=True
                            )
                        # single-chunk (latency-bound) wavefronts: split out
                        # the o-gate sigmoid so it overlaps the DVE cn work
                        if nck == 1:
                            nc.scalar.activation(
                                G[:, :, 0:3, :ck], P[:, :, 0:3, :ck], AF.Sigmoid
                            )
                            nc.scalar.activation(
                                G[:, :, 3, :ck], P[:, :, 3, :ck], AF.Sigmoid
                            )
                        else:
                            nc.scalar.activation(
                                G[:, :, 0:4, :ck], P[:, :, 0:4, :ck], AF.Sigmoid
                            )
                        return G

                    def cn_stage(k, cr0, cr1, ck, co1, G):
                        sl = sn * 2 + k
                        pw = slice(co1, co1 + ck)
                        T1 = wpool.tile([O, 2, CKMAX], f16, tag=f"t1{sl}")
                        T2 = wpool.tile([O, 2, CKMAX], f16, tag=f"t2{sl}")
                        # g = 2*sigmoid(2x) - 1 affine fix, in place
                        nc.vector.tensor_scalar(
                            G[:, :, 0, :ck],
                            G[:, :, 0, :ck],
                            2.0,
                            -1.0,
                            ALU.mult,
                            ALU.add,
                        )
                        # t2 = i*g (both branches in one op)
                        nc.vector.tensor_tensor(
                            T2[:, :, :ck], G[:, :, 1, :ck], G[:, :, 0, :ck], ALU.mult
                        )
                        # t1 = f*c_pred, both branches via the overlapping view
                        nc.vector.tensor_tensor(
                            T1[:, :, :ck],
                            G[:, :, 2, :ck],
                            c_overlap(sn, bp, cr0, ck),
                            ALU.mult,
                        )
                        # cn = t1 + t2 -> CP[0:2]
                        nc.vector.tensor_tensor(
                            CP[:, 0:2, pw], T1[:, :, :ck], T2[:, :, :ck], ALU.add
                        )

                    def tanh_commit(k, cr0, cr1, ck, co1, G):
                        sl = sn * 2 + k
                        pw = slice(co1, co1 + ck)
                        Eh = wpool.tile([O, 2, CKMAX], f16, tag=f"eh{sl}")
                        # tau = tanh(cn)
                        nc.scalar.activation(TAU[:, :, pw], CP[:, 0:2, pw], AF.Tanh)
                        # pp = o*tau -> CP[2:4]; Pool engine when
                        # throughput-bound, DVE when latency-bound
                        pp_eng = nc.gpsimd if d >= 25 else nc.vector
                        pp_eng.tensor_tensor(
                            CP[:, 2:4, pw], G[:, :, 3, :ck], TAU[:, :, pw], ALU.mult
                        )
                        # E = ws0*[cn_t, pp_t] + bias, then
                        # new state [ct | ht] = ws1*[cn_l, pp_l] + E in one op
                        nc.vector.tensor_scalar(
                            Eh[:, :, :ck],
                            CP[:, 0:4:2, pw],
                            ws0_s[:],
                            bias_s[:],
                            ALU.mult,
                            ALU.add,
                        )
                        nc.vector.scalar_tensor_tensor(
                            sc[:, bc, 0:2, cr0 + 1 : cr1 + 1, :],
                            CP[:, 1:4:2, pw],
                            ws1_s[:],
                            Eh[:, :, :ck],
                            ALU.mult,
                            ALU.add,
                        )

                    def dma_out():
                        nc.sync.dma_start(
                            h_diag[:, sn * ncols + off : sn * ncols + off + db],
                            sc[:, bc, 1, i0 + 1 : i1 + 1, :],
                        )

                    return mm_sigma, cn_stage, tanh_commit, dma_out

                ctxs = [scan_ctx(0), scan_ctx(1)]
                if d <= 30:
                    # ramp: emit each scan's wavefront whole (sigmas first
                    # within a scan); the sibling scan's block fills the
                    # chain stalls of this one
                    for sn in range(2):
                        mm, cn, tcm, _ = ctxs[sn]
                        Gs = [mm(*c) for c in cks]
                        for c, G in zip(cks, Gs):
                            cn(*c, G)
                            tcm(*c, G)
                else:
                    # steady: interleave the scans at chunk granularity, the
                    # sibling's mm+cn between a chunk's cn and its tanh
                    for c in cks:
                        Gs = {}
                        for sn in range(2):
                            Gs[sn] = ctxs[sn][0](*c)
                            ctxs[sn][1](*c, Gs[sn])
                        for sn in range(2):
                            ctxs[sn][2](*c, Gs[sn])
                for sn in range(2):
                    ctxs[sn][3]()

    nc.compile()
    return nc


# ---------------------------------------------------------------- host side


def _diag_index(h, w):
    cells = []
    for t, i0, i1, _ in _wavefronts(h, w, SB):
        for i in range(i0, i1):
            cells.append((i, t - i))
    return np.array(cells)


def _prep_core_inputs(inputs, core, h, w):
    flips = [(False, False), (False, True), (True, False), (True, True)]
    d = core // 2
    fy, fx = flips[d]
    cells = _diag_index(h, w)

    x_diag = np.ones((CIN + 1, 2 * h * w * SB), np.float16)
    for sn in range(2):
        q = (core % 2) * 2 + sn
        xd = inputs["x"][q * SB : (q + 1) * SB]  # (SB, CIN, H, W)
        if fy:
            xd = xd[:, :, ::-1, :]
        if fx:
            xd = xd[:, :, :, ::-1]
        x_hw = np.ascontiguousarray(np.transpose(xd, (1, 2, 3, 0)))  # (CIN,H,W,SB)
        x_cells = x_hw[:, cells[:, 0], cells[:, 1], :].reshape(CIN, h * w * SB)
        x_diag[:CIN, sn * h * w * SB : (sn + 1) * h * w * SB] = x_cells.astype(
            np.float16
        )

    # gate order [i, f, o, g]
    gw_h = [inputs["w_hi"][d], inputs["w_hf"][d], inputs["w_ho"][d], inputs["w_hg"][d]]
    gw_x = [inputs["w_ii"][d], inputs["w_if"][d], inputs["w_io"][d], inputs["w_ig"][d]]
    gb = [inputs["b_i"][d], inputs["b_f"][d], inputs["b_o"][d], inputs["b_g"][d]]

    # gate g (block 3) weights doubled: kernel computes tanh via 2*sig(2x)-1
    whT = np.concatenate(
        [wh.T * (2.0 if g == 3 else 1.0) for g, wh in enumerate(gw_h)], axis=1
    ).astype(np.float16)
    wxT = np.zeros((CIN + 1, NG * O), np.float16)
    for g in range(NG):
        s = 2.0 if g == 3 else 1.0
        wxT[:CIN, g * O : (g + 1) * O] = (gw_x[g].T * s).astype(np.float16)
        wxT[CIN, g * O : (g + 1) * O] = (gb[g] * s).astype(np.float16)

    ws = inputs["weighted_sum"][d]
    return {
        "x_diag": x_diag,
        "whT": whT,
        "wxT": wxT,
        "ws0v": np.full((O, 1), ws[0], np.float32),
        "ws1v": np.full((O, 1), ws[1], np.float32),
        "biasv": np.asarray(inputs["bias"][d], np.float32).reshape(O, 1),
    }


def _assemble_output(results, h, w):
    flips = [(False, False), (False, True), (True, False), (True, True)]
    cells = _diag_index(h, w)
    inv = np.empty(h * w, np.int64)
    inv[cells[:, 0] * w + cells[:, 1]] = np.arange(h * w)

    out = np.empty((NG, O, B_FULL, h, w), np.float32)
    for core in range(N_CORES):
        d = core // 2
        fy, fx = flips[d]
        hd = results[core]["h_diag"].astype(np.float32)
        for sn in range(2):
            q = (core % 2) * 2 + sn
            hq = hd[:, sn * h * w * SB : (sn + 1) * h * w * SB]
            hv = hq.reshape(O, h * w, SB)[:, inv, :].reshape(O, h, w, SB)
            if fy:
                hv = hv[:, ::-1, :, :]
            if fx:
                hv = hv[:, :, ::-1, :]
            out[d, :, q * SB : (q + 1) * SB] = np.transpose(hv, (0, 3, 1, 2))
    return out


_module_cache = {}


def _get_module(h=H, w=W):
    key = (h, w)
    if key not in _module_cache:
        _module_cache[key] = build_module(h, w)
    return _module_cache[key]


def make_in_maps(inputs, h=H, w=W):
    return [_prep_core_inputs(inputs, core, h, w) for core in range(N_CORES)]


def kernel(**inputs) -> np.ndarray:
    from concourse import bass_utils

    nc = _get_module(H, W)
    in_maps = make_in_maps(inputs)
    res = bass_utils.run_bass_kernel_spmd(nc, in_maps, core_ids=list(range(N_CORES)))
    return _assemble_output(res.results, H, W)


# revision 26
# speedup vs baseline: 1.5421x; 1.0041x over previous
"""MDLSTM (4-direction 2D-LSTM) Trainium2 kernel, v4.

Sharding: 8 cores = 4 scan directions x 2 batch-quarter PAIRS. Each
core runs TWO independent B=8 scans of the same direction in lockstep
(skew 0). The ramp wavefronts are latency-chain-bound, not work-bound;
two half-width scans ramp concurrently on independent dependency
chains, so the ramp wall-clock is one half-width chain instead of one
full-width chain, while steady state keeps the same instruction sizes
(4 x 128-column chunk-slots per step).

Per-scan wavefront structure (anti-diagonals; cells (i,j) with i+j=t):
  - fp16 end to end; tanh(g) via 2*sigmoid(2x)-1 with weights
    pre-doubled on host.
  - PSUM plane layout [branch][gate]; per gate the two branches'
    accumulation groups sit in different banks.
  - both branches' f*c products in ONE tensor_tensor via an
    overlapping strided AP on the state tile (branch1 = branch0 + B).
  - pp = o*tau on the Pool engine in steady state (DVE relief).
  - ramp wavefronts (d<=30) emit scan-sequentially with all sigmas
    before the first tanh (idle ScalE never head-of-line stalls on the
    DVE's cn); steady wavefronts interleave the two scans at chunk
    granularity with per-chunk tanh + immediate commits (commit(k)
    gates the next wavefront's matmuls, so it must land early).
  - zero-state by Pool memset; geometric x DMA chunks so the first
    wavefronts start early.
"""

import numpy as np

B_FULL, CIN, H, W = 32, 16, 32, 128
O = 128
SB = 8  # batch per scan (two scans per core)
N_CORES = 8
NG = 4  # gates i, f, o, g


def _wavefronts(h, w, b):
    out = []
    off = 0
    for t in range(h + w - 1):
        i0 = max(0, t - (w - 1))
        i1 = min(h, t + 1)
        out.append((t, i0, i1, off))
        off += (i1 - i0) * b
    return out


def _chunk_rows(d):
    if d <= 7:
        return [d]
    a = -(-d // 2)
    return [a, d - a]


def build_module(h, w):
    import bass_rust
    import concourse.bacc as bacc
    import concourse.mybir as mybir
    import concourse.tile as tile

    dt = mybir.dt
    f16 = dt.float16
    f32 = dt.float32
    AF = mybir.ActivationFunctionType
    ALU = mybir.AluOpType

    wfs = _wavefronts(h, w, SB)
    ncols = h * w * SB
    nslots = h + 1

    nc = bacc.Bacc("TRN2", target_bir_lowering=False, debug=False)

    x_diag = nc.dram_tensor("x_diag", [CIN + 1, 2 * ncols], f16, kind="ExternalInput")
    whT = nc.dram_tensor("whT", [O, NG * O], f16, kind="ExternalInput")
    wxT = nc.dram_tensor("wxT", [CIN + 1, NG * O], f16, kind="ExternalInput")
    ws0v = nc.dram_tensor("ws0v", [O, 1], f32, kind="ExternalInput")
    ws1v = nc.dram_tensor("ws1v", [O, 1], f32, kind="ExternalInput")
    biasv = nc.dram_tensor("biasv", [O, 1], f32, kind="ExternalInput")
    h_diag = nc.dram_tensor("h_diag", [O, 2 * ncols], f16, kind="ExternalOutput")

    with tile.TileContext(nc) as tc:
        with (
            tc.tile_pool(name="const", bufs=1) as cpool,
            tc.tile_pool(name="state", bufs=1) as spool,
            tc.tile_pool(name="gates", bufs=2) as gpool,
            tc.tile_pool(name="work", bufs=2) as wpool,
            tc.tile_pool(name="psum", bufs=1, space="PSUM") as ppool,
        ):
            whT_s = cpool.tile([O, NG * O], f16, tag="whT")
            wxT_s = cpool.tile([CIN + 1, NG * O], f16, tag="wxT")
            ws0_s = cpool.tile([O, 1], f32, tag="ws0")
            ws1_s = cpool.tile([O, 1], f32, tag="ws1")
            bias_s = cpool.tile([O, 1], f32, tag="bias")
            nc.sync.dma_start(whT_s[:], whT[:])
            nc.sync.dma_start(wxT_s[:], wxT[:])
            nc.sync.dma_start(ws0_s[:], ws0v[:])
            nc.sync.dma_start(ws1_s[:], ws1v[:])
            nc.sync.dma_start(bias_s[:], biasv[:])

            # per-scan state: [buf][c|h][slot][b]; slot 0 stays zero forever.
            # Zeroed by memset on the (otherwise idle) Pool engine.
            scs = []
            for sn in range(2):
                t_ = spool.tile([O, 2, 2, nslots, SB], f16, tag=f"sc{sn}")
                nc.gpsimd.memset(t_[:], 0.0)
                scs.append(t_)

            # both scans' x resident in SBUF; geometric DMA chunks per scan
            # (interleaved) so both scans' first wavefronts start in ~2us
            xs = cpool.tile([CIN + 1, 2 * ncols], f16, tag="xs")
            bounds = [0, 512, 1536, 3584, 8192, 16384, 24576, ncols]
            for lo, hi in zip(bounds, bounds[1:]):
                for sn in range(2):
                    b0 = sn * ncols
                    nc.sync.dma_start(
                        xs[:, b0 + lo : b0 + hi], x_diag[:, b0 + lo : b0 + hi]
                    )

            # 4 chunk-slots (2 per scan) of [O, 2, 4, 128] f32 = 4KB each
            # fill the 16KB PSUM exactly
            CKMAX = 16 * SB

            sc_pstride = 2 * 2 * nslots * SB

            def c_overlap(sn, bp, r0, ck):
                # [O, 2, ck] view of c-state: branch0 at slots r0.., branch1
                # shifted one slot (+SB elements); reads overlap on purpose.
                base = scs[sn][:, bp, 0, r0, 0]
                return bass_rust.AP(
                    base.tensor, base.offset, [[sc_pstride, O], [SB, 2], [1, ck]]
                )

            # gate order [i, f, o, g]; psum/gate plane = 2*gi + branch
            GI, GF, GO, GG = 0, 1, 2, 3
            plane_of = {GG: 0, GI: 1, GF: 2, GO: 3}

            for t, i0, i1, off in wfs:
                d = i1 - i0
                bp = (t + 1) % 2  # prev state buffer
                bc = t % 2
                db = d * SB
                rows = _chunk_rows(d)
                nck = len(rows)

                # chunk metadata: (k, row0, row1, ck, wavefront col offset)
                cks = []
                r0, o1 = i0, 0
                for k, dk in enumerate(rows):
                    cks.append((k, r0, r0 + dk, dk * SB, o1))
                    r0 += dk
                    o1 += dk * SB

                # Big wavefronts interleave sigma/tanh per chunk (backlogged
                # ScalE never stalls); latency-bound mid-ramp wavefronts
                # issue both sigmas before the first tanh.
                reorder = nck == 2 and d <= 30

                def scan_ctx(sn):
                    sc = scs[sn]
                    xbase = sn * ncols + off
                    CP = wpool.tile([O, 4, 2 * CKMAX], f16, tag=f"cp{sn}")
                    TAU = wpool.tile([O, 2, 2 * CKMAX], f16, tag=f"tau{sn}")

                    def mm_sigma(k, cr0, cr1, ck, co1):
                        sl = sn * 2 + k
                        xr = xs[:, xbase + co1 : xbase + co1 + ck]
                        rhs_t = sc[:, bp, 1, cr0:cr1, :]
                        rhs_l = sc[:, bp, 1, cr0 + 1 : cr1 + 1, :]
                        P = ppool.tile([O, 2, 4, CKMAX], f32, tag=f"p{sl}")
                        G = gpool.tile([O, 2, 4, CKMAX], f16, tag=f"g{sl}")
                        for g in (GG, GI, GF, GO):
                            p = plane_of[g]
                            lx = wxT_s[:, g * O : (g + 1) * O]
                            lw = whT_s[:, g * O : (g + 1) * O]
                            nc.tensor.matmul(
                                P[:, 0, p, :ck], lx, xr, start=True, stop=False
                            )
                            nc.tensor.matmul(
                                P[:, 1, p, :ck], lx, xr, start=True, stop=False
                            )
                            nc.tensor.matmul(
                                P[:, 0, p, :ck], lw, rhs_t, start=False, stop=True
                            )
                            nc.tensor.matmul(
                                P[:, 1, p, :ck], lw, rhs_l, start=False, stop=True
                            )
                        # single-chunk (latency-bound) wavefronts: split out
                        # the o-gate sigmoid so it overlaps the DVE cn work
                        if nck == 1:
                            nc.scalar.activation(
                                G[:, :, 0:3, :ck], P[:, :, 0:3, :ck], AF.Sigmoid
                            )
                            nc.scalar.activation(
                                G[:, :, 3, :ck], P[:, :, 3, :ck], AF.Sigmoid
                            )
                        else:
                            nc.scalar.activation(
                                G[:, :, 0:4, :ck], P[:, :, 0:4, :ck], AF.Sigmoid
                            )
                        return G

                    def cn_stage(k, cr0, cr1, ck, co1, G):
                        sl = sn * 2 + k
                        pw = slice(co1, co1 + ck)
                        T1 = wpool.tile([O, 2, CKMAX], f16, tag=f"t1{sl}")
                        T2 = wpool.tile([O, 2, CKMAX], f16, tag=f"t2{sl}")
                        # g = 2*sigmoid(2x) - 1 affine fix, in place
                        nc.vector.tensor_scalar(
                            G[:, :, 0, :ck],
                            G[:, :, 0, :ck],
                            2.0,
                            -1.0,
                            ALU.mult,
                            ALU.add,
                        )
                        # t2 = i*g (both branches in one op)
                        nc.vector.tensor_tensor(
                            T2[:, :, :ck], G[:, :, 1, :ck], G[:, :, 0, :ck], ALU.mult
                        )
                        # t1 = f*c_pred, both branches via the overlapping view
                        nc.vector.tensor_tensor(
                            T1[:, :, :ck],
                            G[:, :, 2, :ck],
                            c_overlap(sn, bp, cr0, ck),
                            ALU.mult,
                        )
                        # cn = t1 + t2 -> CP[0:2]
                        nc.vector.tensor_tensor(
                            CP[:, 0:2, pw], T1[:, :, :ck], T2[:, :, :ck], ALU.add
                        )

                    def tanh_commit(k, cr0, cr1, ck, co1, G):
                        sl = sn * 2 + k
                        pw = slice(co1, co1 + ck)
                        Eh = wpool.tile([O, 2, CKMAX], f16, tag=f"eh{sl}")
                        # tau = tanh(cn)
                        nc.scalar.activation(TAU[:, :, pw], CP[:, 0:2, pw], AF.Tanh)
                        # pp = o*tau -> CP[2:4]; Pool engine when
                        # throughput-bound, DVE when latency-bound
                        pp_eng = nc.gpsimd if d >= 31 else nc.vector
                        pp_eng.tensor_tensor(
                            CP[:, 2:4, pw], G[:, :, 3, :ck], TAU[:, :, pw], ALU.mult
                        )
                        # E = ws0*[cn_t, pp_t] + bias, then
                        # new state [ct | ht] = ws1*[cn_l, pp_l] + E in one op
                        nc.vector.tensor_scalar(
                            Eh[:, :, :ck],
                            CP[:, 0:4:2, pw],
                            ws0_s[:],
                            bias_s[:],
                            ALU.mult,
                            ALU.add,
                        )
                        nc.vector.scalar_tensor_tensor(
                            sc[:, bc, 0:2, cr0 + 1 : cr1 + 1, :],
                            CP[:, 1:4:2, pw],
                            ws1_s[:],
                            Eh[:, :, :ck],
                            ALU.mult,
                            ALU.add,
                        )

                    def dma_out():
                        nc.sync.dma_start(
                            h_diag[:, sn * ncols + off : sn * ncols + off + db],
                            sc[:, bc, 1, i0 + 1 : i1 + 1, :],
                        )

                    return mm_sigma, cn_stage, tanh_commit, dma_out

                ctxs = [scan_ctx(0), scan_ctx(1)]
                if d <= 30:
                    # ramp: emit each scan's wavefront whole (sigmas first
                    # within a scan); the sibling scan's block fills the
                    # chain stalls of this one
                    for sn in range(2):
                        mm, cn, tcm, _ = ctxs[sn]
                        Gs = [mm(*c) for c in cks]
                        for c, G in zip(cks, Gs):
                            cn(*c, G)
                            tcm(*c, G)
                else:
                    # steady: interleave the scans at chunk granularity, the
                    # sibling's mm+cn between a chunk's cn and its tanh
                    for c in cks:
                        Gs = {}
                        for sn in range(2):
                            Gs[sn] = ctxs[sn][0](*c)
                            ctxs[sn][1](*c, Gs[sn])
                        for sn in range(2):
                            ctxs[sn][2](*c, Gs[sn])
                for sn in range(2):
                    ctxs[sn][3]()

    nc.compile()
    return nc


# ---------------------------------------------------------------- host side


def _diag_index(h, w):
    cells = []
    for t, i0, i1, _ in _wavefronts(h, w, SB):
        for i in range(i0, i1):
            cells.append((i, t - i))
    return np.array(cells)


def _prep_core_inputs(inputs, core, h, w):
    flips = [(False, False), (False, True), (True, False), (True, True)]
    d = core // 2
    fy, fx = flips[d]
    cells = _diag_index(h, w)

    x_diag = np.ones((CIN + 1, 2 * h * w * SB), np.float16)
    for sn in range(2):
        q = (core % 2) * 2 + sn
        xd = inputs["x"][q * SB : (q + 1) * SB]  # (SB, CIN, H, W)
        if fy:
            xd = xd[:, :, ::-1, :]
        if fx:
            xd = xd[:, :, :, ::-1]
        x_hw = np.ascontiguousarray(np.transpose(xd, (1, 2, 3, 0)))  # (CIN,H,W,SB)
        x_cells = x_hw[:, cells[:, 0], cells[:, 1], :].reshape(CIN, h * w * SB)
        x_diag[:CIN, sn * h * w * SB : (sn + 1) * h * w * SB] = x_cells.astype(
            np.float16
        )

    # gate order [i, f, o, g]
    gw_h = [inputs["w_hi"][d], inputs["w_hf"][d], inputs["w_ho"][d], inputs["w_hg"][d]]
    gw_x = [inputs["w_ii"][d], inputs["w_if"][d], inputs["w_io"][d], inputs["w_ig"][d]]
    gb = [inputs["b_i"][d], inputs["b_f"][d], inputs["b_o"][d], inputs["b_g"][d]]

    # gate g (block 3) weights doubled: kernel computes tanh via 2*sig(2x)-1
    whT = np.concatenate(
        [wh.T * (2.0 if g == 3 else 1.0) for g, wh in enumerate(gw_h)], axis=1
    ).astype(np.float16)
    wxT = np.zeros((CIN + 1, NG * O), np.float16)
    for g in range(NG):
        s = 2.0 if g == 3 else 1.0
        wxT[:CIN, g * O : (g + 1) * O] = (gw_x[g].T * s).astype(np.float16)
        wxT[CIN, g * O : (g + 1) * O] = (gb[g] * s).astype(np.float16)

    ws = inputs["weighted_sum"][d]
    return {
        "x_diag": x_diag,
        "whT": whT,
        "wxT": wxT,
        "ws0v": np.full((O, 1), ws[0], np.float32),
        "ws1v": np.full((O, 1), ws[1], np.float32),
        "biasv": np.asarray(inputs["bias"][d], np.float32).reshape(O, 1),
    }


def _assemble_output(results, h, w):
    flips = [(False, False), (False, True), (True, False), (True, True)]
    cells = _diag_index(h, w)
    inv = np.empty(h * w, np.int64)
    inv[cells[:, 0] * w + cells[:, 1]] = np.arange(h * w)

    out = np.empty((NG, O, B_FULL, h, w), np.float32)
    for core in range(N_CORES):
        d = core // 2
        fy, fx = flips[d]
        hd = results[core]["h_diag"].astype(np.float32)
        for sn in range(2):
            q = (core % 2) * 2 + sn
            hq = hd[:, sn * h * w * SB : (sn + 1) * h * w * SB]
            hv = hq.reshape(O, h * w, SB)[:, inv, :].reshape(O, h, w, SB)
            if fy:
                hv = hv[:, ::-1, :, :]
            if fx:
                hv = hv[:, :, ::-1, :]
            out[d, :, q * SB : (q + 1) * SB] = np.transpose(hv, (0, 3, 1, 2))
    return out


_module_cache = {}


def _get_module(h=H, w=W):
    key = (h, w)
    if key not in _module_cache:
        _module_cache[key] = build_module(h, w)
    return _module_cache[key]


def make_in_maps(inputs, h=H, w=W):
    return [_prep_core_inputs(inputs, core, h, w) for core in range(N_CORES)]


def kernel(**inputs) -> np.ndarray:
    from concourse import bass_utils

    nc = _get_module(H, W)
    in_maps = make_in_maps(inputs)
    res = bass_utils.run_bass_kernel_spmd(nc, in_maps, core_ids=list(range(N_CORES)))
    return _assemble_output(res.results, H, W)


# revision 27
# speedup vs baseline: 1.5446x; 1.0016x over previous
"""MDLSTM (4-direction 2D-LSTM) Trainium2 kernel, v4.

Sharding: 8 cores = 4 scan directions x 2 batch-quarter PAIRS. Each
core runs TWO independent B=8 scans of the same direction in lockstep
(skew 0). The ramp wavefronts are latency-chain-bound, not work-bound;
two half-width scans ramp concurrently on independent dependency
chains, so the ramp wall-clock is one half-width chain instead of one
full-width chain, while steady state keeps the same instruction sizes
(4 x 128-column chunk-slots per step).

Per-scan wavefront structure (anti-diagonals; cells (i,j) with i+j=t):
  - fp16 end to end; tanh(g) via 2*sigmoid(2x)-1 with weights
    pre-doubled on host.
  - PSUM plane layout [branch][gate]; per gate the two branches'
    accumulation groups sit in different banks.
  - both branches' f*c products in ONE tensor_tensor via an
    overlapping strided AP on the state tile (branch1 = branch0 + B).
  - pp = o*tau on the Pool engine in steady state (DVE relief).
  - ramp wavefronts (d<=30) emit scan-sequentially with all sigmas
    before the first tanh (idle ScalE never head-of-line stalls on the
    DVE's cn); steady wavefronts interleave the two scans at chunk
    granularity with per-chunk tanh + immediate commits (commit(k)
    gates the next wavefront's matmuls, so it must land early).
  - zero-state by Pool memset; geometric x DMA chunks so the first
    wavefronts start early.
"""

import numpy as np

B_FULL, CIN, H, W = 32, 16, 32, 128
O = 128
SB = 8  # batch per scan (two scans per core)
N_CORES = 8
NG = 4  # gates i, f, o, g


def _wavefronts(h, w, b):
    out = []
    off = 0
    for t in range(h + w - 1):
        i0 = max(0, t - (w - 1))
        i1 = min(h, t + 1)
        out.append((t, i0, i1, off))
        off += (i1 - i0) * b
    return out


def _chunk_rows(d):
    if d <= 7:
        return [d]
    a = -(-d // 2)
    return [a, d - a]


def build_module(h, w):
    import bass_rust
    import concourse.bacc as bacc
    import concourse.mybir as mybir
    import concourse.tile as tile

    dt = mybir.dt
    f16 = dt.float16
    f32 = dt.float32
    AF = mybir.ActivationFunctionType
    ALU = mybir.AluOpType

    wfs = _wavefronts(h, w, SB)
    ncols = h * w * SB
    nslots = h + 1

    nc = bacc.Bacc("TRN2", target_bir_lowering=False, debug=False)

    x_diag = nc.dram_tensor("x_diag", [CIN + 1, 2 * ncols], f16, kind="ExternalInput")
    whT = nc.dram_tensor("whT", [O, NG * O], f16, kind="ExternalInput")
    wxT = nc.dram_tensor("wxT", [CIN + 1, NG * O], f16, kind="ExternalInput")
    ws0v = nc.dram_tensor("ws0v", [O, 1], f32, kind="ExternalInput")
    ws1v = nc.dram_tensor("ws1v", [O, 1], f32, kind="ExternalInput")
    biasv = nc.dram_tensor("biasv", [O, 1], f32, kind="ExternalInput")
    h_diag = nc.dram_tensor("h_diag", [O, 2 * ncols], f16, kind="ExternalOutput")

    with tile.TileContext(nc) as tc:
        with (
            tc.tile_pool(name="const", bufs=1) as cpool,
            tc.tile_pool(name="state", bufs=1) as spool,
            tc.tile_pool(name="gates", bufs=3) as gpool,
            tc.tile_pool(name="work", bufs=2) as wpool,
            tc.tile_pool(name="psum", bufs=1, space="PSUM") as ppool,
        ):
            whT_s = cpool.tile([O, NG * O], f16, tag="whT")
            wxT_s = cpool.tile([CIN + 1, NG * O], f16, tag="wxT")
            ws0_s = cpool.tile([O, 1], f32, tag="ws0")
            ws1_s = cpool.tile([O, 1], f32, tag="ws1")
            bias_s = cpool.tile([O, 1], f32, tag="bias")
            nc.sync.dma_start(whT_s[:], whT[:])
            nc.sync.dma_start(wxT_s[:], wxT[:])
            nc.sync.dma_start(ws0_s[:], ws0v[:])
            nc.sync.dma_start(ws1_s[:], ws1v[:])
            nc.sync.dma_start(bias_s[:], biasv[:])

            # per-scan state: [buf][c|h][slot][b]; slot 0 stays zero forever.
            # Zeroed by memset on the (otherwise idle) Pool engine.
            scs = []
            for sn in range(2):
                t_ = spool.tile([O, 2, 2, nslots, SB], f16, tag=f"sc{sn}")
                nc.gpsimd.memset(t_[:], 0.0)
                scs.append(t_)

            # both scans' x resident in SBUF; geometric DMA chunks per scan
            # (interleaved) so both scans' first wavefronts start in ~2us
            xs = cpool.tile([CIN + 1, 2 * ncols], f16, tag="xs")
            bounds = [0, 512, 1536, 3584, 8192, 16384, 24576, ncols]
            for lo, hi in zip(bounds, bounds[1:]):
                for sn in range(2):
                    b0 = sn * ncols
                    nc.sync.dma_start(
                        xs[:, b0 + lo : b0 + hi], x_diag[:, b0 + lo : b0 + hi]
                    )

            # 4 chunk-slots (2 per scan) of [O, 2, 4, 128] f32 = 4KB each
            # fill the 16KB PSUM exactly
            CKMAX = 16 * SB

            sc_pstride = 2 * 2 * nslots * SB

            def c_overlap(sn, bp, r0, ck):
                # [O, 2, ck] view of c-state: branch0 at slots r0.., branch1
                # shifted one slot (+SB elements); reads overlap on purpose.
                base = scs[sn][:, bp, 0, r0, 0]
                return bass_rust.AP(
                    base.tensor, base.offset, [[sc_pstride, O], [SB, 2], [1, ck]]
                )

            # gate order [i, f, o, g]; psum/gate plane = 2*gi + branch
            GI, GF, GO, GG = 0, 1, 2, 3
            plane_of = {GG: 0, GI: 1, GF: 2, GO: 3}

            for t, i0, i1, off in wfs:
                d = i1 - i0
                bp = (t + 1) % 2  # prev state buffer
                bc = t % 2
                db = d * SB
                rows = _chunk_rows(d)
                nck = len(rows)

                # chunk metadata: (k, row0, row1, ck, wavefront col offset)
                cks = []
                r0, o1 = i0, 0
                for k, dk in enumerate(rows):
                    cks.append((k, r0, r0 + dk, dk * SB, o1))
                    r0 += dk
                    o1 += dk * SB

                # Big wavefronts interleave sigma/tanh per chunk (backlogged
                # ScalE never stalls); latency-bound mid-ramp wavefronts
                # issue both sigmas before the first tanh.
                reorder = nck == 2 and d <= 30

                def scan_ctx(sn):
                    sc = scs[sn]
                    xbase = sn * ncols + off
                    CP = wpool.tile([O, 4, 2 * CKMAX], f16, tag=f"cp{sn}")
                    TAU = wpool.tile([O, 2, 2 * CKMAX], f16, tag=f"tau{sn}")

                    def mm_sigma(k, cr0, cr1, ck, co1):
                        sl = sn * 2 + k
                        xr = xs[:, xbase + co1 : xbase + co1 + ck]
                        rhs_t = sc[:, bp, 1, cr0:cr1, :]
                        rhs_l = sc[:, bp, 1, cr0 + 1 : cr1 + 1, :]
                        P = ppool.tile([O, 2, 4, CKMAX], f32, tag=f"p{sl}")
                        G = gpool.tile([O, 2, 4, CKMAX], f16, tag=f"g{sl}")
                        for g in (GG, GI, GF, GO):
                            p = plane_of[g]
                            lx = wxT_s[:, g * O : (g + 1) * O]
                            lw = whT_s[:, g * O : (g + 1) * O]
                            nc.tensor.matmul(
                                P[:, 0, p, :ck], lx, xr, start=True, stop=False
                            )
                            nc.tensor.matmul(
                                P[:, 1, p, :ck], lx, xr, start=True, stop=False
                            )
                            nc.tensor.matmul(
                                P[:, 0, p, :ck], lw, rhs_t, start=False, stop=True
                            )
                            nc.tensor.matmul(
                                P[:, 1, p, :ck], lw, rhs_l, start=False, stop=True
                            )
                        # single-chunk (latency-bound) wavefronts: split out
                        # the o-gate sigmoid so it overlaps the DVE cn work
                        if nck == 1:
                            nc.scalar.activation(
                                G[:, :, 0:3, :ck], P[:, :, 0:3, :ck], AF.Sigmoid
                            )
                            nc.scalar.activation(
                                G[:, :, 3, :ck], P[:, :, 3, :ck], AF.Sigmoid
                            )
                        else:
                            nc.scalar.activation(
                                G[:, :, 0:4, :ck], P[:, :, 0:4, :ck], AF.Sigmoid
                            )
                        return G

                    def cn_stage(k, cr0, cr1, ck, co1, G):
                        sl = sn * 2 + k
                        pw = slice(co1, co1 + ck)
                        T1 = wpool.tile([O, 2, CKMAX], f16, tag=f"t1{sl}")
                        T2 = wpool.tile([O, 2, CKMAX], f16, tag=f"t2{sl}")
                        # g = 2*sigmoid(2x) - 1 affine fix, in place
                        nc.vector.tensor_scalar(
                            G[:, :, 0, :ck],
                            G[:, :, 0, :ck],
                            2.0,
                            -1.0,
                            ALU.mult,
                            ALU.add,
                        )
                        # t2 = i*g (both branches in one op)
                        nc.vector.tensor_tensor(
                            T2[:, :, :ck], G[:, :, 1, :ck], G[:, :, 0, :ck], ALU.mult
                        )
                        # t1 = f*c_pred, both branches via the overlapping view
                        nc.vector.tensor_tensor(
                            T1[:, :, :ck],
                            G[:, :, 2, :ck],
                            c_overlap(sn, bp, cr0, ck),
                            ALU.mult,
                        )
                        # cn = t1 + t2 -> CP[0:2]
                        nc.vector.tensor_tensor(
                            CP[:, 0:2, pw], T1[:, :, :ck], T2[:, :, :ck], ALU.add
                        )

                    def tanh_commit(k, cr0, cr1, ck, co1, G):
                        sl = sn * 2 + k
                        pw = slice(co1, co1 + ck)
                        Eh = wpool.tile([O, 2, CKMAX], f16, tag=f"eh{sl}")
                        # tau = tanh(cn)
                        nc.scalar.activation(TAU[:, :, pw], CP[:, 0:2, pw], AF.Tanh)
                        # pp = o*tau -> CP[2:4]; Pool engine when
                        # throughput-bound, DVE when latency-bound
                        pp_eng = nc.gpsimd if d >= 31 else nc.vector
                        pp_eng.tensor_tensor(
                            CP[:, 2:4, pw], G[:, :, 3, :ck], TAU[:, :, pw], ALU.mult
                        )
                        # E = ws0*[cn_t, pp_t] + bias, then
                        # new state [ct | ht] = ws1*[cn_l, pp_l] + E in one op
                        nc.vector.tensor_scalar(
                            Eh[:, :, :ck],
                            CP[:, 0:4:2, pw],
                            ws0_s[:],
                            bias_s[:],
                            ALU.mult,
                            ALU.add,
                        )
                        nc.vector.scalar_tensor_tensor(
                            sc[:, bc, 0:2, cr0 + 1 : cr1 + 1, :],
                            CP[:, 1:4:2, pw],
                            ws1_s[:],
                            Eh[:, :, :ck],
                            ALU.mult,
                            ALU.add,
                        )

                    def dma_out():
                        nc.sync.dma_start(
                            h_diag[:, sn * ncols + off : sn * ncols + off + db],
                            sc[:, bc, 1, i0 + 1 : i1 + 1, :],
                        )

                    return mm_sigma, cn_stage, tanh_commit, dma_out

                ctxs = [scan_ctx(0), scan_ctx(1)]
                if d <= 30:
                    # ramp: emit each scan's wavefront whole (sigmas first
                    # within a scan); the sibling scan's block fills the
                    # chain stalls of this one
                    for sn in range(2):
                        mm, cn, tcm, _ = ctxs[sn]
                        Gs = [mm(*c) for c in cks]
                        for c, G in zip(cks, Gs):
                            cn(*c, G)
                            tcm(*c, G)
                else:
                    # steady: interleave the scans at chunk granularity, the
                    # sibling's mm+cn between a chunk's cn and its tanh
                    for c in cks:
                        Gs = {}
                        for sn in range(2):
                            Gs[sn] = ctxs[sn][0](*c)
                            ctxs[sn][1](*c, Gs[sn])
                        for sn in range(2):
                            ctxs[sn][2](*c, Gs[sn])
                for sn in range(2):
                    ctxs[sn][3]()

    nc.compile()
    return nc


# ---------------------------------------------------------------- host side


def _diag_index(h, w):
    cells = []
    for t, i0, i1, _ in _wavefronts(h, w, SB):
        for i in range(i0, i1):
            cells.append((i, t - i))
    return np.array(cells)


def _prep_core_inputs(inputs, core, h, w):
    flips = [(False, False), (False, True), (True, False), (True, True)]
    d = core // 2
    fy, fx = flips[d]
    cells = _diag_index(h, w)

    x_diag = np.ones((CIN + 1, 2 * h * w * SB), np.float16)
    for sn in range(2):
        q = (core % 2) * 2 + sn
        xd = inputs["x"][q * SB : (q + 1) * SB]  # (SB, CIN, H, W)
        if fy:
            xd = xd[:, :, ::-1, :]
        if fx:
            xd = xd[:, :, :, ::-1]
        x_hw = np.ascontiguousarray(np.transpose(xd, (1, 2, 3, 0)))  # (CIN,H,W,SB)
        x_cells = x_hw[:, cells[:, 0], cells[:, 1], :].reshape(CIN, h * w * SB)
        x_diag[:CIN, sn * h * w * SB : (sn + 1) * h * w * SB] = x_cells.astype(
            np.float16
        )

    # gate order [i, f, o, g]
    gw_h = [inputs["w_hi"][d], inputs["w_hf"][d], inputs["w_ho"][d], inputs["w_hg"][d]]
    gw_x = [inputs["w_ii"][d], inputs["w_if"][d], inputs["w_io"][d], inputs["w_ig"][d]]
    gb = [inputs["b_i"][d], inputs["b_f"][d], inputs["b_o"][d], inputs["b_g"][d]]

    # gate g (block 3) weights doubled: kernel computes tanh via 2*sig(2x)-1
    whT = np.concatenate(
        [wh.T * (2.0 if g == 3 else 1.0) for g, wh in enumerate(gw_h)], axis=1
    ).astype(np.float16)
    wxT = np.zeros((CIN + 1, NG * O), np.float16)
    for g in range(NG):
        s = 2.0 if g == 3 else 1.0
        wxT[:CIN, g * O : (g + 1) * O] = (gw_x[g].T * s).astype(np.float16)
        wxT[CIN, g * O : (g + 1) * O] = (gb[g] * s).astype(np.float16)

    ws = inputs["weighted_sum"][d]
    return {
        "x_diag": x_diag,
        "whT": whT,
        "wxT": wxT,
        "ws0v": np.full((O, 1), ws[0], np.float32),
        "ws1v": np.full((O, 1), ws[1], np.float32),
        "biasv": np.asarray(inputs["bias"][d], np.float32).reshape(O, 1),
    }


def _assemble_output(results, h, w):
    flips = [(False, False), (False, True), (True, False), (True, True)]
    cells = _diag_index(h, w)
    inv = np.empty(h * w, np.int64)
    inv[cells[:, 0] * w + cells[:, 1]] = np.arange(h * w)

    out = np.empty((NG, O, B_FULL, h, w), np.float32)
    for core in range(N_CORES):
        d = core // 2
        fy, fx = flips[d]
        hd = results[core]["h_diag"].astype(np.float32)
        for sn in range(2):
            q = (core % 2) * 2 + sn
            hq = hd[:, sn * h * w * SB : (sn + 1) * h * w * SB]
            hv = hq.reshape(O, h * w, SB)[:, inv, :].reshape(O, h, w, SB)
            if fy:
                hv = hv[:, ::-1, :, :]
            if fx:
                hv = hv[:, :, ::-1, :]
            out[d, :, q * SB : (q + 1) * SB] = np.transpose(hv, (0, 3, 1, 2))
    return out


_module_cache = {}


def _get_module(h=H, w=W):
    key = (h, w)
    if key not in _module_cache:
        _module_cache[key] = build_module(h, w)
    return _module_cache[key]


def make_in_maps(inputs, h=H, w=W):
    return [_prep_core_inputs(inputs, core, h, w) for core in range(N_CORES)]


def kernel(**inputs) -> np.ndarray:
    from concourse import bass_utils

    nc = _get_module(H, W)
    in_maps = make_in_maps(inputs)
    res = bass_utils.run_bass_kernel_spmd(nc, in_maps, core_ids=list(range(N_CORES)))
    return _assemble_output(res.results, H, W)


# revision 32
# speedup vs baseline: 1.5651x; 1.0133x over previous
"""MDLSTM (4-direction 2D-LSTM) Trainium2 kernel, v4.

Sharding: 8 cores = 4 scan directions x 2 batch-quarter PAIRS. Each
core runs TWO independent B=8 scans of the same direction in lockstep
(skew 0). The ramp wavefronts are latency-chain-bound, not work-bound;
two half-width scans ramp concurrently on independent dependency
chains, so the ramp wall-clock is one half-width chain instead of one
full-width chain, while steady state keeps the same instruction sizes
(4 x 128-column chunk-slots per step).

Per-scan wavefront structure (anti-diagonals; cells (i,j) with i+j=t):
  - fp16 end to end; tanh(g) via 2*sigmoid(2x)-1 with weights
    pre-doubled on host.
  - PSUM plane layout [branch][gate]; per gate the two branches'
    accumulation groups sit in different banks.
  - both branches' f*c products in ONE tensor_tensor via an
    overlapping strided AP on the state tile (branch1 = branch0 + B).
  - pp = o*tau on the Pool engine in steady state (DVE relief).
  - ramp wavefronts (d<=30) emit scan-sequentially with all sigmas
    before the first tanh (idle ScalE never head-of-line stalls on the
    DVE's cn); steady wavefronts interleave the two scans at chunk
    granularity with per-chunk tanh + immediate commits (commit(k)
    gates the next wavefront's matmuls, so it must land early).
  - ramp wavefronts put the SMALLER chunk first so commit(0) lands
    earlier and unblocks the next wavefront's matmuls sooner.
  - x uploaded via Pool-engine (SWDGE) DMAs, bypassing the single
    shared HWDGE device the weight/output DMAs serialize on; tiny
    frontier state memsets and the first x chunks are queued ahead of
    the bulk state memsets so wavefront 0 starts at ~2.5us.
"""

import numpy as np

B_FULL, CIN, H, W = 32, 16, 32, 128
O = 128
SB = 8  # batch per scan (two scans per core)
N_CORES = 8
NG = 4  # gates i, f, o, g


def _wavefronts(h, w, b):
    out = []
    off = 0
    for t in range(h + w - 1):
        i0 = max(0, t - (w - 1))
        i1 = min(h, t + 1)
        out.append((t, i0, i1, off))
        off += (i1 - i0) * b
    return out


def _chunk_rows(d):
    if d <= 7:
        return [d]
    a = -(-d // 2)
    return [a, d - a]


def build_module(h, w):
    import bass_rust
    import concourse.bacc as bacc
    import concourse.mybir as mybir
    import concourse.tile as tile

    dt = mybir.dt
    f16 = dt.float16
    f32 = dt.float32
    AF = mybir.ActivationFunctionType
    ALU = mybir.AluOpType

    wfs = _wavefronts(h, w, SB)
    ncols = h * w * SB
    nslots = h + 1

    nc = bacc.Bacc("TRN2", target_bir_lowering=False, debug=False)

    x_diag = nc.dram_tensor("x_diag", [CIN + 1, 2 * ncols], f16, kind="ExternalInput")
    whT = nc.dram_tensor("whT", [O, NG * O], f16, kind="ExternalInput")
    wxT = nc.dram_tensor("wxT", [CIN + 1, NG * O], f16, kind="ExternalInput")
    ws0v = nc.dram_tensor("ws0v", [O, 1], f32, kind="ExternalInput")
    ws1v = nc.dram_tensor("ws1v", [O, 1], f32, kind="ExternalInput")
    biasv = nc.dram_tensor("biasv", [O, 1], f32, kind="ExternalInput")
    h_diag = nc.dram_tensor("h_diag", [O, 2 * ncols], f16, kind="ExternalOutput")

    with tile.TileContext(nc) as tc:
        with (
            tc.tile_pool(name="const", bufs=1) as cpool,
            tc.tile_pool(name="state", bufs=1) as spool,
            tc.tile_pool(name="gates", bufs=3) as gpool,
            tc.tile_pool(name="work", bufs=2) as wpool,
            tc.tile_pool(name="psum", bufs=1, space="PSUM") as ppool,
        ):
            whT_s = cpool.tile([O, NG * O], f16, tag="whT")
            wxT_s = cpool.tile([CIN + 1, NG * O], f16, tag="wxT")
            ws0_s = cpool.tile([O, 1], f32, tag="ws0")
            ws1_s = cpool.tile([O, 1], f32, tag="ws1")
            bias_s = cpool.tile([O, 1], f32, tag="bias")
            nc.sync.dma_start(wxT_s[:], wxT[:])
            nc.sync.dma_start(whT_s[:], whT[:])
            nc.sync.dma_start(ws0_s[:], ws0v[:])
            nc.sync.dma_start(ws1_s[:], ws1v[:])
            nc.sync.dma_start(bias_s[:], biasv[:])

            # per-scan state: [buf][c|h][slot][b]; slot 0 stays zero forever.
            # Zeroed by memset on the (otherwise idle) Pool engine.
            scs = []
            for sn in range(2):
                t_ = spool.tile([O, 2, 2, nslots, SB], f16, tag=f"sc{sn}")
                scs.append(t_)

            # Pool queue order matters at startup: tiny frontier memsets and
            # the first x chunks first (wavefront 0 needs only slots 0-2 and
            # a few x columns), then the bulk state memsets and the rest of
            # x. Pool DMAs go through SWDGE, bypassing the single shared
            # HWDGE device that the weight/output DMAs serialize on.
            xs = cpool.tile([CIN + 1, 2 * ncols], f16, tag="xs")
            bounds = [0, 512, 1536, 3584, 8192, 16384, 24576, ncols]

            def x_dma(sn, lo, hi):
                b0 = sn * ncols
                nc.gpsimd.dma_start(
                    xs[:, b0 + lo : b0 + hi], x_diag[:, b0 + lo : b0 + hi]
                )

            for sn in range(2):
                nc.gpsimd.memset(scs[sn][:, :, :, 0:3, :], 0.0)
            for sn in range(2):
                x_dma(sn, 0, bounds[1])
            for sn in range(2):
                nc.gpsimd.memset(scs[sn][:, :, :, 3:, :], 0.0)
            for lo, hi in zip(bounds[1:], bounds[2:]):
                for sn in range(2):
                    x_dma(sn, lo, hi)

            # 4 chunk-slots (2 per scan) of [O, 2, 4, 128] f32 = 4KB each
            # fill the 16KB PSUM exactly
            CKMAX = 16 * SB

            sc_pstride = 2 * 2 * nslots * SB

            def c_overlap(sn, bp, r0, ck):
                # [O, 2, ck] view of c-state: branch0 at slots r0.., branch1
                # shifted one slot (+SB elements); reads overlap on purpose.
                base = scs[sn][:, bp, 0, r0, 0]
                return bass_rust.AP(
                    base.tensor, base.offset, [[sc_pstride, O], [SB, 2], [1, ck]]
                )

            # gate order [i, f, o, g]; psum/gate plane = 2*gi + branch
            GI, GF, GO, GG = 0, 1, 2, 3
            plane_of = {GG: 0, GI: 1, GF: 2, GO: 3}

            for t, i0, i1, off in wfs:
                d = i1 - i0
                bp = (t + 1) % 2  # prev state buffer
                bc = t % 2
                db = d * SB
                rows = _chunk_rows(d)
                if t >= 127 and len(rows) == 2:
                    rows = rows[::-1]  # smaller chunk first on the down-ramp
                nck = len(rows)

                # chunk metadata: (k, row0, row1, ck, wavefront col offset)
                cks = []
                r0, o1 = i0, 0
                for k, dk in enumerate(rows):
                    cks.append((k, r0, r0 + dk, dk * SB, o1))
                    r0 += dk
                    o1 += dk * SB

                # Big wavefronts interleave sigma/tanh per chunk (backlogged
                # ScalE never stalls); latency-bound mid-ramp wavefronts
                # issue both sigmas before the first tanh.
                reorder = nck == 2 and d <= 30

                def scan_ctx(sn):
                    sc = scs[sn]
                    xbase = sn * ncols + off
                    CP = wpool.tile([O, 4, 2 * CKMAX], f16, tag=f"cp{sn}")
                    TAU = wpool.tile([O, 2, 2 * CKMAX], f16, tag=f"tau{sn}")

                    def mm_sigma(k, cr0, cr1, ck, co1):
                        sl = sn * 2 + k
                        xr = xs[:, xbase + co1 : xbase + co1 + ck]
                        rhs_t = sc[:, bp, 1, cr0:cr1, :]
                        rhs_l = sc[:, bp, 1, cr0 + 1 : cr1 + 1, :]
                        P = ppool.tile([O, 2, 4, CKMAX], f32, tag=f"p{sl}")
                        G = gpool.tile([O, 2, 4, CKMAX], f16, tag=f"g{sl}")
                        for g in (GG, GI, GF, GO):
                            p = plane_of[g]
                            lx = wxT_s[:, g * O : (g + 1) * O]
                            lw = whT_s[:, g * O : (g + 1) * O]
                            nc.tensor.matmul(
                                P[:, 0, p, :ck], lx, xr, start=True, stop=False
                            )
                            nc.tensor.matmul(
                                P[:, 1, p, :ck], lx, xr, start=True, stop=False
                            )
                            nc.tensor.matmul(
                                P[:, 0, p, :ck], lw, rhs_t, start=False, stop=True
                            )
                            nc.tensor.matmul(
                                P[:, 1, p, :ck], lw, rhs_l, start=False, stop=True
                            )
                        # single-chunk (latency-bound) wavefronts: split out
                        # the o-gate sigmoid so it overlaps the DVE cn work
                        if nck == 1:
                            nc.scalar.activation(
                                G[:, :, 0:3, :ck], P[:, :, 0:3, :ck], AF.Sigmoid
                            )
                            nc.scalar.activation(
                                G[:, :, 3, :ck], P[:, :, 3, :ck], AF.Sigmoid
                            )
                        else:
                            nc.scalar.activation(
                                G[:, :, 0:4, :ck], P[:, :, 0:4, :ck], AF.Sigmoid
                            )
                        return G

                    def cn_stage(k, cr0, cr1, ck, co1, G):
                        sl = sn * 2 + k
                        pw = slice(co1, co1 + ck)
                        T1 = wpool.tile([O, 2, CKMAX], f16, tag=f"t1{sl}")
                        T2 = wpool.tile([O, 2, CKMAX], f16, tag=f"t2{sl}")
                        # g = 2*sigmoid(2x) - 1 affine fix, in place
                        nc.vector.tensor_scalar(
                            G[:, :, 0, :ck],
                            G[:, :, 0, :ck],
                            2.0,
                            -1.0,
                            ALU.mult,
                            ALU.add,
                        )
                        # t2 = i*g (both branches in one op)
                        nc.vector.tensor_tensor(
                            T2[:, :, :ck], G[:, :, 1, :ck], G[:, :, 0, :ck], ALU.mult
                        )
                        # t1 = f*c_pred, both branches via the overlapping view
                        nc.vector.tensor_tensor(
                            T1[:, :, :ck],
                            G[:, :, 2, :ck],
                            c_overlap(sn, bp, cr0, ck),
                            ALU.mult,
                        )
                        # cn = t1 + t2 -> CP[0:2]
                        nc.vector.tensor_tensor(
                            CP[:, 0:2, pw], T1[:, :, :ck], T2[:, :, :ck], ALU.add
                        )

                    def tanh_commit(k, cr0, cr1, ck, co1, G):
                        sl = sn * 2 + k
                        pw = slice(co1, co1 + ck)
                        Eh = wpool.tile([O, 2, CKMAX], f16, tag=f"eh{sl}")
                        # tau = tanh(cn)
                        nc.scalar.activation(TAU[:, :, pw], CP[:, 0:2, pw], AF.Tanh)
                        # pp = o*tau -> CP[2:4]; Pool engine when
                        # throughput-bound, DVE when latency-bound
                        pp_eng = nc.gpsimd if d >= 31 else nc.vector
                        pp_eng.tensor_tensor(
                            CP[:, 2:4, pw], G[:, :, 3, :ck], TAU[:, :, pw], ALU.mult
                        )
                        # E = ws0*[cn_t, pp_t] + bias, then
                        # new state [ct | ht] = ws1*[cn_l, pp_l] + E in one op
                        nc.vector.tensor_scalar(
                            Eh[:, :, :ck],
                            CP[:, 0:4:2, pw],
                            ws0_s[:],
                            bias_s[:],
                            ALU.mult,
                            ALU.add,
                        )
                        nc.vector.scalar_tensor_tensor(
                            sc[:, bc, 0:2, cr0 + 1 : cr1 + 1, :],
                            CP[:, 1:4:2, pw],
                            ws1_s[:],
                            Eh[:, :, :ck],
                            ALU.mult,
                            ALU.add,
                        )

                    def dma_out():
                        nc.sync.dma_start(
                            h_diag[:, sn * ncols + off : sn * ncols + off + db],
                            sc[:, bc, 1, i0 + 1 : i1 + 1, :],
                        )

                    return mm_sigma, cn_stage, tanh_commit, dma_out

                ctxs = [scan_ctx(0), scan_ctx(1)]
                if d <= 30:
                    # ramp: emit each scan's wavefront whole (sigmas first
                    # within a scan); the sibling scan's block fills the
                    # chain stalls of this one
                    for sn in range(2):
                        mm, cn, tcm, _ = ctxs[sn]
                        Gs = [mm(*c) for c in cks]
                        for c, G in zip(cks, Gs):
                            cn(*c, G)
                            tcm(*c, G)
                else:
                    # steady: interleave the scans at chunk granularity, the
                    # sibling's mm+cn between a chunk's cn and its tanh
                    for c in cks:
                        order = ((t + c[0]) % 2, 1 - (t + c[0]) % 2)
                        Gs = {}
                        for sn in order:
                            Gs[sn] = ctxs[sn][0](*c)
                            ctxs[sn][1](*c, Gs[sn])
                        for sn in order:
                            ctxs[sn][2](*c, Gs[sn])
                for sn in range(2):
                    ctxs[sn][3]()

    nc.compile()
    return nc


# ---------------------------------------------------------------- host side


def _diag_index(h, w):
    cells = []
    for t, i0, i1, _ in _wavefronts(h, w, SB):
        for i in range(i0, i1):
            cells.append((i, t - i))
    return np.array(cells)


def _prep_core_inputs(inputs, core, h, w):
    flips = [(False, False), (False, True), (True, False), (True, True)]
    d = core // 2
    fy, fx = flips[d]
    cells = _diag_index(h, w)

    x_diag = np.ones((CIN + 1, 2 * h * w * SB), np.float16)
    for sn in range(2):
        q = (core % 2) * 2 + sn
        xd = inputs["x"][q * SB : (q + 1) * SB]  # (SB, CIN, H, W)
        if fy:
            xd = xd[:, :, ::-1, :]
        if fx:
            xd = xd[:, :, :, ::-1]
        x_hw = np.ascontiguousarray(np.transpose(xd, (1, 2, 3, 0)))  # (CIN,H,W,SB)
        x_cells = x_hw[:, cells[:, 0], cells[:, 1], :].reshape(CIN, h * w * SB)
        x_diag[:CIN, sn * h * w * SB : (sn + 1) * h * w * SB] = x_cells.astype(
            np.float16
        )

    # gate order [i, f, o, g]
    gw_h = [inputs["w_hi"][d], inputs["w_hf"][d], inputs["w_ho"][d], inputs["w_hg"][d]]
    gw_x = [inputs["w_ii"][d], inputs["w_if"][d], inputs["w_io"][d], inputs["w_ig"][d]]
    gb = [inputs["b_i"][d], inputs["b_f"][d], inputs["b_o"][d], inputs["b_g"][d]]

    # gate g (block 3) weights doubled: kernel computes tanh via 2*sig(2x)-1
    whT = np.concatenate(
        [wh.T * (2.0 if g == 3 else 1.0) for g, wh in enumerate(gw_h)], axis=1
    ).astype(np.float16)
    wxT = np.zeros((CIN + 1, NG * O), np.float16)
    for g in range(NG):
        s = 2.0 if g == 3 else 1.0
        wxT[:CIN, g * O : (g + 1) * O] = (gw_x[g].T * s).astype(np.float16)
        wxT[CIN, g * O : (g + 1) * O] = (gb[g] * s).astype(np.float16)

    ws = inputs["weighted_sum"][d]
    return {
        "x_diag": x_diag,
        "whT": whT,
        "wxT": wxT,
        "ws0v": np.full((O, 1), ws[0], np.float32),
        "ws1v": np.full((O, 1), ws[1], np.float32),
        "biasv": np.asarray(inputs["bias"][d], np.float32).reshape(O, 1),
    }


def _assemble_output(results, h, w):
    flips = [(False, False), (False, True), (True, False), (True, True)]
    cells = _diag_index(h, w)
    inv = np.empty(h * w, np.int64)
    inv[cells[:, 0] * w + cells[:, 1]] = np.arange(h * w)

    out = np.empty((NG, O, B_FULL, h, w), np.float32)
    for core in range(N_CORES):
        d = core // 2
        fy, fx = flips[d]
        hd = results[core]["h_diag"].astype(np.float32)
        for sn in range(2):
            q = (core % 2) * 2 + sn
            hq = hd[:, sn * h * w * SB : (sn + 1) * h * w * SB]
            hv = hq.reshape(O, h * w, SB)[:, inv, :].reshape(O, h, w, SB)
            if fy:
                hv = hv[:, ::-1, :, :]
            if fx:
                hv = hv[:, :, ::-1, :]
            out[d, :, q * SB : (q + 1) * SB] = np.transpose(hv, (0, 3, 1, 2))
    return out


_module_cache = {}


def _get_module(h=H, w=W):
    key = (h, w)
    if key not in _module_cache:
        _module_cache[key] = build_module(h, w)
    return _module_cache[key]


def make_in_maps(inputs, h=H, w=W):
    return [_prep_core_inputs(inputs, core, h, w) for core in range(N_CORES)]


def kernel(**inputs) -> np.ndarray:
    from concourse import bass_utils

    nc = _get_module(H, W)
    in_maps = make_in_maps(inputs)
    res = bass_utils.run_bass_kernel_spmd(nc, in_maps, core_ids=list(range(N_CORES)))
    return _assemble_output(res.results, H, W)


# revision 34
# speedup vs baseline: 1.5901x; 1.0160x over previous
"""MDLSTM (4-direction 2D-LSTM) Trainium2 kernel, v4.

Sharding: 8 cores = 4 scan directions x 2 batch-quarter PAIRS. Each
core runs TWO independent B=8 scans of the same direction in lockstep
(skew 0). The ramp wavefronts are latency-chain-bound, not work-bound;
two half-width scans ramp concurrently on independent dependency
chains, so the ramp wall-clock is one half-width chain instead of one
full-width chain, while steady state keeps the same instruction sizes
(4 x 128-column chunk-slots per step).

Per-scan wavefront structure (anti-diagonals; cells (i,j) with i+j=t):
  - fp16 end to end; tanh(g) via 2*sigmoid(2x)-1 with weights
    pre-doubled on host.
  - PSUM plane layout [branch][gate]; per gate the two branches'
    accumulation groups sit in different banks.
  - both branches' f*c products in ONE tensor_tensor via an
    overlapping strided AP on the state tile (branch1 = branch0 + B).
  - pp = o*tau on the Pool engine in steady state (DVE relief).
  - ramp wavefronts (d<=30) emit scan-sequentially with all sigmas
    before the first tanh (idle ScalE never head-of-line stalls on the
    DVE's cn); steady wavefronts interleave the two scans at chunk
    granularity with per-chunk tanh + immediate commits (commit(k)
    gates the next wavefront's matmuls, so it must land early).
  - ramp wavefronts put the SMALLER chunk first so commit(0) lands
    earlier and unblocks the next wavefront's matmuls sooner; steady
    wavefronts alternate which scan is emitted first per (step+chunk)
    so neither scan's tail is systematically last.
  - x uploaded via Pool-engine (SWDGE) DMAs, bypassing the single
    shared HWDGE device the weight/output DMAs serialize on; tiny
    frontier state memsets and the first x chunks are queued ahead of
    the bulk state memsets so wavefront 0 starts at ~2.5us.
"""

import numpy as np

B_FULL, CIN, H, W = 32, 16, 32, 128
O = 128
SB = 8  # batch per scan (two scans per core)
N_CORES = 8
NG = 4  # gates i, f, o, g


def _wavefronts(h, w, b):
    out = []
    off = 0
    for t in range(h + w - 1):
        i0 = max(0, t - (w - 1))
        i1 = min(h, t + 1)
        out.append((t, i0, i1, off))
        off += (i1 - i0) * b
    return out


def _chunk_rows(d):
    if d <= 7:
        return [d]
    a = -(-d // 2)
    return [a, d - a]


def build_module(h, w):
    import bass_rust
    import concourse.bacc as bacc
    import concourse.mybir as mybir
    import concourse.tile as tile

    dt = mybir.dt
    f16 = dt.float16
    f32 = dt.float32
    AF = mybir.ActivationFunctionType
    ALU = mybir.AluOpType

    wfs = _wavefronts(h, w, SB)
    ncols = h * w * SB
    nslots = h + 1

    nc = bacc.Bacc("TRN2", target_bir_lowering=False, debug=False)

    x_diag = nc.dram_tensor("x_diag", [CIN + 1, 2 * ncols], f16, kind="ExternalInput")
    whT = nc.dram_tensor("whT", [O, NG * O], f16, kind="ExternalInput")
    wxT = nc.dram_tensor("wxT", [CIN + 1, NG * O], f16, kind="ExternalInput")
    ws0v = nc.dram_tensor("ws0v", [O, 1], f32, kind="ExternalInput")
    ws1v = nc.dram_tensor("ws1v", [O, 1], f32, kind="ExternalInput")
    biasv = nc.dram_tensor("biasv", [O, 1], f32, kind="ExternalInput")
    h_diag = nc.dram_tensor("h_diag", [O, 2 * ncols], f16, kind="ExternalOutput")

    with tile.TileContext(nc) as tc:
        with (
            tc.tile_pool(name="const", bufs=1) as cpool,
            tc.tile_pool(name="state", bufs=1) as spool,
            tc.tile_pool(name="gates", bufs=3) as gpool,
            tc.tile_pool(name="work", bufs=2) as wpool,
            tc.tile_pool(name="psum", bufs=1, space="PSUM") as ppool,
        ):
            whT_s = cpool.tile([O, NG * O], f16, tag="whT")
            wxT_s = cpool.tile([CIN + 1, NG * O], f16, tag="wxT")
            ws0_s = cpool.tile([O, 1], f32, tag="ws0")
            ws1_s = cpool.tile([O, 1], f32, tag="ws1")
            bias_s = cpool.tile([O, 1], f32, tag="bias")
            nc.sync.dma_start(wxT_s[:], wxT[:])
            nc.sync.dma_start(whT_s[:], whT[:])
            nc.sync.dma_start(ws0_s[:], ws0v[:])
            nc.sync.dma_start(ws1_s[:], ws1v[:])
            nc.sync.dma_start(bias_s[:], biasv[:])

            # per-scan state: [buf][c|h][slot][b]; slot 0 stays zero forever.
            # Zeroed by memset on the (otherwise idle) Pool engine.
            scs = []
            for sn in range(2):
                t_ = spool.tile([O, 2, 2, nslots, SB], f16, tag=f"sc{sn}")
                scs.append(t_)

            # Pool queue order matters at startup: tiny frontier memsets and
            # the first x chunks first (wavefront 0 needs only slots 0-2 and
            # a few x columns), then the bulk state memsets and the rest of
            # x. Pool DMAs go through SWDGE, bypassing the single shared
            # HWDGE device that the weight/output DMAs serialize on.
            xs = cpool.tile([CIN + 1, 2 * ncols], f16, tag="xs")
            bounds = [0, 512, 1536, 3584, 8192, 16384, 24576, ncols]

            def x_dma(sn, lo, hi):
                b0 = sn * ncols
                nc.gpsimd.dma_start(
                    xs[:, b0 + lo : b0 + hi], x_diag[:, b0 + lo : b0 + hi]
                )

            for sn in range(2):
                nc.gpsimd.memset(scs[sn][:, :, :, 0:3, :], 0.0)
            for sn in range(2):
                x_dma(sn, 0, bounds[1])
            for sn in range(2):
                nc.gpsimd.memset(scs[sn][:, :, :, 3:, :], 0.0)
            for lo, hi in zip(bounds[1:], bounds[2:]):
                for sn in range(2):
                    x_dma(sn, lo, hi)

            # 4 chunk-slots (2 per scan) of [O, 2, 4, 128] f32 = 4KB each
            # fill the 16KB PSUM exactly
            CKMAX = 16 * SB

            sc_pstride = 2 * 2 * nslots * SB

            def c_overlap(sn, bp, r0, ck):
                # [O, 2, ck] view of c-state: branch0 at slots r0.., branch1
                # shifted one slot (+SB elements); reads overlap on purpose.
                base = scs[sn][:, bp, 0, r0, 0]
                return bass_rust.AP(
                    base.tensor, base.offset, [[sc_pstride, O], [SB, 2], [1, ck]]
                )

            # gate order [i, f, o, g]; psum/gate plane = 2*gi + branch
            GI, GF, GO, GG = 0, 1, 2, 3
            plane_of = {GG: 0, GI: 1, GF: 2, GO: 3}

            for t, i0, i1, off in wfs:
                d = i1 - i0
                bp = (t + 1) % 2  # prev state buffer
                bc = t % 2
                db = d * SB
                rows = _chunk_rows(d)
                if t >= 127 and len(rows) == 2:
                    rows = rows[::-1]  # smaller chunk first on the down-ramp
                nck = len(rows)

                # chunk metadata: (k, row0, row1, ck, wavefront col offset)
                cks = []
                r0, o1 = i0, 0
                for k, dk in enumerate(rows):
                    cks.append((k, r0, r0 + dk, dk * SB, o1))
                    r0 += dk
                    o1 += dk * SB

                # Big wavefronts interleave sigma/tanh per chunk (backlogged
                # ScalE never stalls); latency-bound mid-ramp wavefronts
                # issue both sigmas before the first tanh.
                reorder = nck == 2 and d <= 30

                def scan_ctx(sn):
                    sc = scs[sn]
                    xbase = sn * ncols + off
                    CP = wpool.tile([O, 4, 2 * CKMAX], f16, tag=f"cp{sn}")
                    TAU = wpool.tile([O, 2, 2 * CKMAX], f16, tag=f"tau{sn}")

                    def mm_sigma(k, cr0, cr1, ck, co1):
                        sl = sn * 2 + k
                        xr = xs[:, xbase + co1 : xbase + co1 + ck]
                        rhs_t = sc[:, bp, 1, cr0:cr1, :]
                        rhs_l = sc[:, bp, 1, cr0 + 1 : cr1 + 1, :]
                        P = ppool.tile([O, 2, 4, CKMAX], f32, tag=f"p{sl}")
                        G = gpool.tile([O, 2, 4, CKMAX], f16, tag=f"g{sl}")
                        for g in (GG, GI, GF, GO):
                            p = plane_of[g]
                            lx = wxT_s[:, g * O : (g + 1) * O]
                            lw = whT_s[:, g * O : (g + 1) * O]
                            nc.tensor.matmul(
                                P[:, 0, p, :ck], lx, xr, start=True, stop=False
                            )
                            nc.tensor.matmul(
                                P[:, 1, p, :ck], lx, xr, start=True, stop=False
                            )
                            nc.tensor.matmul(
                                P[:, 0, p, :ck], lw, rhs_t, start=False, stop=True
                            )
                            nc.tensor.matmul(
                                P[:, 1, p, :ck], lw, rhs_l, start=False, stop=True
                            )
                        # single-chunk (latency-bound) wavefronts: split out
                        # the o-gate sigmoid so it overlaps the DVE cn work
                        if nck == 1:
                            nc.scalar.activation(
                                G[:, :, 0:3, :ck], P[:, :, 0:3, :ck], AF.Sigmoid
                            )
                            nc.scalar.activation(
                                G[:, :, 3, :ck], P[:, :, 3, :ck], AF.Sigmoid
                            )
                        else:
                            nc.scalar.activation(
                                G[:, :, 0:4, :ck], P[:, :, 0:4, :ck], AF.Sigmoid
                            )
                        return G

                    def cn_stage(k, cr0, cr1, ck, co1, G):
                        sl = sn * 2 + k
                        pw = slice(co1, co1 + ck)
                        T1 = wpool.tile([O, 2, CKMAX], f16, tag=f"t1{sl}")
                        T2 = wpool.tile([O, 2, CKMAX], f16, tag=f"t2{sl}")
                        # g = 2*sigmoid(2x) - 1 affine fix, in place
                        nc.vector.tensor_scalar(
                            G[:, :, 0, :ck],
                            G[:, :, 0, :ck],
                            2.0,
                            -1.0,
                            ALU.mult,
                            ALU.add,
                        )
                        # t2 = i*g (both branches in one op)
                        nc.vector.tensor_tensor(
                            T2[:, :, :ck], G[:, :, 1, :ck], G[:, :, 0, :ck], ALU.mult
                        )
                        # t1 = f*c_pred, both branches via the overlapping view
                        nc.vector.tensor_tensor(
                            T1[:, :, :ck],
                            G[:, :, 2, :ck],
                            c_overlap(sn, bp, cr0, ck),
                            ALU.mult,
                        )
                        # cn = t1 + t2 -> CP[0:2]
                        nc.vector.tensor_tensor(
                            CP[:, 0:2, pw], T1[:, :, :ck], T2[:, :, :ck], ALU.add
                        )

                    def tanh_commit(k, cr0, cr1, ck, co1, G):
                        sl = sn * 2 + k
                        pw = slice(co1, co1 + ck)
                        Eh = wpool.tile([O, 2, CKMAX], f16, tag=f"eh{sl}")
                        # tau = tanh(cn)
                        nc.scalar.activation(TAU[:, :, pw], CP[:, 0:2, pw], AF.Tanh)
                        # pp = o*tau -> CP[2:4]; Pool engine when
                        # throughput-bound, DVE when latency-bound
                        pp_eng = nc.gpsimd if d >= 31 else nc.vector
                        pp_eng.tensor_tensor(
                            CP[:, 2:4, pw], G[:, :, 3, :ck], TAU[:, :, pw], ALU.mult
                        )
                        # E = ws0*[cn_t, pp_t] + bias, then
                        # new state [ct | ht] = ws1*[cn_l, pp_l] + E in one op
                        nc.vector.tensor_scalar(
                            Eh[:, :, :ck],
                            CP[:, 0:4:2, pw],
                            ws0_s[:],
                            bias_s[:],
                            ALU.mult,
                            ALU.add,
                        )
                        nc.vector.scalar_tensor_tensor(
                            sc[:, bc, 0:2, cr0 + 1 : cr1 + 1, :],
                            CP[:, 1:4:2, pw],
                            ws1_s[:],
                            Eh[:, :, :ck],
                            ALU.mult,
                            ALU.add,
                        )

                    def dma_out():
                        nc.sync.dma_start(
                            h_diag[:, sn * ncols + off : sn * ncols + off + db],
                            sc[:, bc, 1, i0 + 1 : i1 + 1, :],
                        )

                    return mm_sigma, cn_stage, tanh_commit, dma_out

                ctxs = [scan_ctx(0), scan_ctx(1)]
                if d <= 30:
                    # ramp: emit each scan's wavefront whole (sigmas first
                    # within a scan); the sibling scan's block fills the
                    # chain stalls of this one
                    for sn in range(2):
                        mm, cn, tcm, _ = ctxs[sn]
                        Gs = [mm(*c) for c in cks]
                        for c, G in zip(cks, Gs):
                            cn(*c, G)
                            tcm(*c, G)
                else:
                    # steady: interleave the scans at chunk granularity, the
                    # sibling's mm+cn between a chunk's cn and its tanh
                    for c in cks:
                        order = (c[0] % 2, 1 - c[0] % 2)
                        Gs = {}
                        for sn in order:
                            Gs[sn] = ctxs[sn][0](*c)
                            ctxs[sn][1](*c, Gs[sn])
                        for sn in order:
                            ctxs[sn][2](*c, Gs[sn])
                for sn in range(2):
                    ctxs[sn][3]()

    nc.compile()
    return nc


# ---------------------------------------------------------------- host side


def _diag_index(h, w):
    cells = []
    for t, i0, i1, _ in _wavefronts(h, w, SB):
        for i in range(i0, i1):
            cells.append((i, t - i))
    return np.array(cells)


def _prep_core_inputs(inputs, core, h, w):
    flips = [(False, False), (False, True), (True, False), (True, True)]
    d = core // 2
    fy, fx = flips[d]
    cells = _diag_index(h, w)

    x_diag = np.ones((CIN + 1, 2 * h * w * SB), np.float16)
    for sn in range(2):
        q = (core % 2) * 2 + sn
        xd = inputs["x"][q * SB : (q + 1) * SB]  # (SB, CIN, H, W)
        if fy:
            xd = xd[:, :, ::-1, :]
        if fx:
            xd = xd[:, :, :, ::-1]
        x_hw = np.ascontiguousarray(np.transpose(xd, (1, 2, 3, 0)))  # (CIN,H,W,SB)
        x_cells = x_hw[:, cells[:, 0], cells[:, 1], :].reshape(CIN, h * w * SB)
        x_diag[:CIN, sn * h * w * SB : (sn + 1) * h * w * SB] = x_cells.astype(
            np.float16
        )

    # gate order [i, f, o, g]
    gw_h = [inputs["w_hi"][d], inputs["w_hf"][d], inputs["w_ho"][d], inputs["w_hg"][d]]
    gw_x = [inputs["w_ii"][d], inputs["w_if"][d], inputs["w_io"][d], inputs["w_ig"][d]]
    gb = [inputs["b_i"][d], inputs["b_f"][d], inputs["b_o"][d], inputs["b_g"][d]]

    # gate g (block 3) weights doubled: kernel computes tanh via 2*sig(2x)-1
    whT = np.concatenate(
        [wh.T * (2.0 if g == 3 else 1.0) for g, wh in enumerate(gw_h)], axis=1
    ).astype(np.float16)
    wxT = np.zeros((CIN + 1, NG * O), np.float16)
    for g in range(NG):
        s = 2.0 if g == 3 else 1.0
        wxT[:CIN, g * O : (g + 1) * O] = (gw_x[g].T * s).astype(np.float16)
        wxT[CIN, g * O : (g + 1) * O] = (gb[g] * s).astype(np.float16)

    ws = inputs["weighted_sum"][d]
    return {
        "x_diag": x_diag,
        "whT": whT,
        "wxT": wxT,
        "ws0v": np.full((O, 1), ws[0], np.float32),
        "ws1v": np.full((O, 1), ws[1], np.float32),
        "biasv": np.asarray(inputs["bias"][d], np.float32).reshape(O, 1),
    }


def _assemble_output(results, h, w):
    flips = [(False, False), (False, True), (True, False), (True, True)]
    cells = _diag_index(h, w)
    inv = np.empty(h * w, np.int64)
    inv[cells[:, 0] * w + cells[:, 1]] = np.arange(h * w)

    out = np.empty((NG, O, B_FULL, h, w), np.float32)
    for core in range(N_CORES):
        d = core // 2
        fy, fx = flips[d]
        hd = results[core]["h_diag"].astype(np.float32)
        for sn in range(2):
            q = (core % 2) * 2 + sn
            hq = hd[:, sn * h * w * SB : (sn + 1) * h * w * SB]
            hv = hq.reshape(O, h * w, SB)[:, inv, :].reshape(O, h, w, SB)
            if fy:
                hv = hv[:, ::-1, :, :]
            if fx:
                hv = hv[:, :, ::-1, :]
            out[d, :, q * SB : (q + 1) * SB] = np.transpose(hv, (0, 3, 1, 2))
    return out


_module_cache = {}


def _get_module(h=H, w=W):
    key = (h, w)
    if key not in _module_cache:
        _module_cache[key] = build_module(h, w)
    return _module_cache[key]


def make_in_maps(inputs, h=H, w=W):
    return [_prep_core_inputs(inputs, core, h, w) for core in range(N_CORES)]


def kernel(**inputs) -> np.ndarray:
    from concourse import bass_utils

    nc = _get_module(H, W)
    in_maps = make_in_maps(inputs)
    res = bass_utils.run_bass_kernel_spmd(nc, in_maps, core_ids=list(range(N_CORES)))
    return _assemble_output(res.results, H, W)
